# revision 17
# baseline (speedup 1.0000x reference)
"""Causal single-head attention (B=4, S=4096, D=1024, H=64) on 8 TRN2 NeuronCores.

Strategy (v4: sequence-parallel, pipelined k/v AllGather)
---------------------------------------------------------
Data-parallel over batch (2 cores per batch element); within a pair the q rows
are split by 128-row block parity (even core: natural blocks 0,2,..,30; odd:
1,3,..,31), which load-balances the causal triangle.  Each core loads only its
own half of x (bf16, 4 MB), projects q/k/v for its rows, and the pair
exchanges k and v with pipelined AllGathers (8 pieces of 2 blocks).  Each core
then computes COMPLETE attention rows for its own q blocks — no output
combine, and the softmax denominator is a per-partition scalar.

Scheduling notes (tuned against the TRN2 timeline cost model):
- matmul cost = moving columns only, and bf16 runs 1 col/cycle at any width;
  everything computes in bf16 (host casts, ~5e-3 rel err vs the 2e-2 gate).
- One DMA bus, FIFO: the sync queue carries params -> x pieces -> the
  stage/readback hops interleaved in exact consumption order, so small
  latency-critical hops are never stuck behind bulk x traffic.  The scalar
  queue carries ONLY exp (a waiting DMA would head-of-line block the
  activation engine); collectives (and the final output writes) live on the
  gpsimd queue.
- The tile framework tracks SBUF deps at tile granularity: wave(w) must be
  emitted after readback(w) but before readback(w+1), or its score matmuls
  would inherit waits on later exchange pieces.
- k/v SBUF slots are indexed by AllGather rank order, so every lhsT offset is
  static; per-core host-computed mask data absorbs the even/odd asymmetry
  (the SPMD instruction stream is identical on all 8 cores).  For q block i
  (local), rank-r slot j holds natural block 2j+r; extents are uniformly
  2i+2 with the j==i slots masked per parity (diagonal triangle / full /
  none) via an extra accumulating matmul lhsT=I, rhs=mask into the scores
  PSUM — the mask costs tensor-engine columns, not vector ops.
- scores^T tiles [128 k, 128 q] are exactly the lhsT layout the out matmul
  wants: out_acc[128 q, 65] += lhsT=exp_tile, rhs=v_nat[128 k, 65] — 65
  moving cols per (k block, q block); v col 64 is constant 1 so the softmax
  denominator accumulates for free, then DVE reciprocal + multiply finish
  each row block.
- The tensor engine p-state ramps only under continuous load (2.4 GHz after
  3us busy, else 1.2); a short warmup plus small per-tile filler matmuls
  (overwritten by the next start=True scores) keep it from bouncing between
  p-states while the activation engine paces the exp stream.
- exp runs on [128, 8*128] PSUM tiles (34 ops); out matmuls trail the exp
  stream by two tiles so the tensor engine never waits on the activation
  engine at a tile boundary.
"""

import numpy as np
from contextlib import ExitStack

import concourse.bass as bass
import concourse.mybir as mybir
import concourse.tile as tile
from concourse import bacc
from concourse.bass_utils import run_bass_kernel_spmd

F32 = mybir.dt.float32
BF16 = mybir.dt.bfloat16

B, S, D, H = 4, 4096, 1024, 64
NCORES = 8
NCH = D // 128        # contraction chunks
NMYB = 16             # my q blocks per core
NPIECE = 8            # exchange pieces, 2 blocks each
MASK_VAL = -8.0e9     # raw-score mask; exp(0.125*(s+MASK_VAL)) == 0
SCALE = 0.125         # 1/sqrt(H)
GPT = 8               # score/exp groups per PSUM tile ([128, GPT*128])
FILL = 2              # filler matmuls (256 cols each) per tile for PE p-state

NK = 64 * 256         # k piece elems in the flat exchange buffer
NV = 128 * 130        # v piece elems (2 blocks x 65 cols)
NX = NK + NV          # = 64*258*2


def build_program(with_cc: bool = True):
    nc = bacc.Bacc(num_devices=NCORES)

    xT = nc.declare_dram_parameter("xT", [D, S // 2], BF16, isOutput=False)
    wall = nc.declare_dram_parameter("wall", [D, 256], BF16, isOutput=False)
    bqk = nc.declare_dram_parameter("bqk", [2 * H, 1], F32, isOutput=False)
    bv = nc.declare_dram_parameter("bv", [1, H], F32, isOutput=False)
    masks = nc.declare_dram_parameter("masks", [2, 128, 128], BF16, isOutput=False)
    ident = nc.declare_dram_parameter("ident", [128, 128], BF16, isOutput=False)
    out = nc.declare_dram_parameter("out", [S // 2, H], F32, isOutput=True)

    xT3 = xT.rearrange("(c p) s -> p c s", p=128)        # [128, 8, 2048]
    wall3 = wall.rearrange("(c p) h -> p c h", p=128)    # [128, 8, 256]
    masks3 = masks.rearrange("m p j -> p m j")           # [128, 2, 128]
    out3 = out.rearrange("(n p) h -> p n h", p=128)      # [128, 16, 64]

    with ExitStack() as ctx:
        tc = ctx.enter_context(tile.TileContext(nc))

        singles = ctx.enter_context(tc.tile_pool(name="singles", bufs=1))
        dram = ctx.enter_context(tc.tile_pool(name="dram", bufs=1, space="DRAM"))

        xt_all = singles.tile([128, NCH, S // 2], BF16)
        wall_sb = singles.tile([128, NCH, 256], BF16)
        bqk_sb = singles.tile([2 * H, 1], F32)
        bv_bc = singles.tile([128, H], F32)
        masks_sb = singles.tile([128, 2, 128], BF16)
        ident_sb = singles.tile([128, 128], BF16)

        # priority order on the sync queue: tiny params, weights, then the x
        # pieces interleaved with the exchange hops
        nc.sync.dma_start(out=ident_sb, in_=ident[:, :])
        nc.sync.dma_start(out=bqk_sb, in_=bqk[:, :])
        nc.sync.dma_start(out=masks_sb, in_=masks3)
        bv_b = bass.AP(tensor=bv[:, :].tensor, offset=bv[:, :].offset,
                       ap=[[0, 128], [1, H]])
        nc.sync.dma_start(out=bv_bc, in_=bv_b)
        nc.sync.dma_start(out=wall_sb, in_=wall3)

        def x_dma(p):
            nc.sync.dma_start(
                out=xt_all[:, :, 256 * p : 256 * p + 256],
                in_=xT3[:, :, 256 * p : 256 * p + 256],
            )

        # persistent SBUF state; k/v slot (r, j) = pair rank r, local block j
        qT_sb = singles.tile([H, NMYB, 128], BF16)
        kT_sb = singles.tile([H, 2, NMYB, 128], BF16)
        v_sb = singles.tile([128, 2, NMYB, 65], BF16)
        kst_all = singles.tile([H, NMYB, 128], BF16)
        vst_all = singles.tile([128, NMYB, 65], BF16)
        out_stage = singles.tile([128, NMYB, H], F32)
        rcp_sb = singles.tile([128, NMYB], F32)
        nc.vector.memset(vst_all[:, :, H : H + 1], 1.0)

        kv_in = [dram.tile([1, NX], BF16, tag=f"kvi{p}", name=f"kv_in{p}")
                 for p in range(NPIECE)]
        kv_red = [dram.tile([2, NX], BF16, tag=f"kvr{p}", name=f"kv_red{p}")
                  for p in range(NPIECE)]

        pj = ctx.enter_context(tc.tile_pool(name="pj", bufs=2, space="PSUM"))
        ps = ctx.enter_context(tc.tile_pool(name="ps", bufs=2, space="PSUM"))
        pacc = ctx.enter_context(tc.tile_pool(name="pacc", bufs=2, space="PSUM"))
        pexp_pool = ctx.enter_context(tc.tile_pool(name="pexp", bufs=4))

        def proj_piece(p):
            """Project q/k/v for my token blocks 2p, 2p+1 and stage them."""
            lo = 256 * p
            b0 = 2 * p
            # [q|k] 0:256 | v_nat b0 256:320 | v_nat b0+1 320:384
            pjt = pj.tile([128, 384], F32, tag="pj", name="pjt")
            for ch in range(NCH):
                nc.tensor.matmul(
                    pjt[:, 0:256], lhsT=wall_sb[:, ch, 0 : 2 * H],
                    rhs=xt_all[:, ch, lo : lo + 256],
                    start=(ch == 0), stop=(ch == NCH - 1),
                )
            for b2 in range(2):
                for ch in range(NCH):
                    nc.tensor.matmul(
                        pjt[:, 256 + 64 * b2 : 320 + 64 * b2],
                        lhsT=xt_all[:, ch, lo + 128 * b2 : lo + 128 * b2 + 128],
                        rhs=wall_sb[:, ch, 2 * H : 3 * H],
                        start=(ch == 0), stop=(ch == NCH - 1),
                    )
            nc.vector.tensor_scalar_add(
                qT_sb[:, b0 : b0 + 2, :], pjt[0:H, 0:256], bqk_sb[0:H, :]
            )
            nc.vector.tensor_scalar_add(
                kst_all[:, b0 : b0 + 2, :], pjt[H : 2 * H, 0:256],
                bqk_sb[H : 2 * H, :],
            )
            nc.vector.tensor_add(vst_all[:, b0, 0:H], pjt[:, 256:320], bv_bc)
            nc.vector.tensor_add(vst_all[:, b0 + 1, 0:H], pjt[:, 320:384], bv_bc)

            k_dst = kv_in[p][:, 0:NK].rearrange("o (q s) -> (o q) s", q=H)
            v_dst = kv_in[p][:, NK:NX].rearrange("o (q s) -> (o q) s", q=128)
            nc.sync.dma_start(out=k_dst, in_=kst_all[:, b0 : b0 + 2, :])
            nc.sync.dma_start(out=v_dst, in_=vst_all[:, b0 : b0 + 2, :])

        def exchange_cc(p):
            """Pair-AllGather of k/v piece p (gpsimd queue)."""
            if with_cc:
                nc.gpsimd.collective_compute(
                    "AllGather",
                    mybir.AluOpType.bypass,
                    replica_groups=[[0, 1], [2, 3], [4, 5], [6, 7]],
                    ins=[kv_in[p][:, :]],
                    outs=[kv_red[p][:, :]],
                )
            else:
                # model the gather as a full-size local copy (258*2B runs)
                src = bass.AP(
                    tensor=kv_in[p][:, :].tensor,
                    offset=kv_in[p][:, :].offset,
                    ap=[[0, 2], [258, 128], [1, 258]],
                )
                nc.gpsimd.dma_start(
                    out=kv_red[p][:, :].rearrange("r (q s) -> r q s", s=258),
                    in_=src,
                )

        def readback(p):
            """Read AllGather piece p back into the rank-indexed k/v slots."""
            k_src = kv_red[p][:, 0:NK].rearrange("r (q s) -> q r s", q=H)
            v_src = kv_red[p][:, NK:NX].rearrange("r (q s) -> q r s", q=128)
            nc.sync.dma_start(
                out=kT_sb[:, :, 2 * p : 2 * p + 2, :], in_=k_src
            )
            nc.sync.dma_start(
                out=v_sb[:, :, 2 * p : 2 * p + 2, :], in_=v_src
            )

        # ---- attention pipeline ----
        cur = {"ps": None, "n": 0, "meta": []}
        deferred = []  # (ps_tile, pexp_tile, meta) awaiting out-matmul emission
        acc_of = {}

        def get_acc(i):
            if i not in acc_of:
                t = pacc.tile([128, 2, 66], F32, tag="pacc", name="acc_t")
                acc_of[i] = t
                acc_of[i + 1] = t
            return acc_of[i]

        def emit_out(batch):
            """Out matmuls (and norms) for a completed exp tile."""
            ps_t, px_t, meta = batch
            for g, (i, r, j) in enumerate(meta):
                nc.tensor.matmul(
                    acc_of[i][:, i % 2, 0:65],
                    lhsT=px_t[:, g, :],
                    rhs=v_sb[:, r, j, :],
                    start=(r == 0 and j == 0),
                    stop=(r == 1 and j == i),
                )
                if r == 1 and j == i:
                    nc.vector.reciprocal(
                        rcp_sb[:, i : i + 1], acc_of[i][:, i % 2, 64:65]
                    )
                    nc.vector.tensor_scalar_mul(
                        out_stage[:, i, :], acc_of[i][:, i % 2, 0:64],
                        rcp_sb[:, i : i + 1],
                    )
                    if i % 4 == 3:
                        g4 = i // 4
                        nc.gpsimd.dma_start(
                            out=out3[:, 4 * g4 : 4 * g4 + 4, :],
                            in_=out_stage[:, 4 * g4 : 4 * g4 + 4, :],
                        )

        def flush_tile():
            """Close the current scores tile: exp it, queue its out matmuls."""
            if cur["n"] == 0:
                return
            ps_t, meta = cur["ps"], cur["meta"]
            px_t = pexp_pool.tile([128, GPT, 128], BF16, tag="px", name="px_t")
            n = cur["n"]
            nc.scalar.activation(
                px_t[:, 0:n, :], ps_t[:, 0:n, :],
                mybir.ActivationFunctionType.Exp, scale=SCALE,
            )
            deferred.append((ps_t, px_t, list(meta)))
            cur["ps"], cur["n"], cur["meta"] = None, 0, []
            if len(deferred) > 2:
                emit_out(deferred.pop(0))

        def group(i, r, j):
            """Scores (+mask) for q block i vs k slot (r, j)."""
            if cur["ps"] is None:
                cur["ps"] = ps.tile([128, GPT, 128], F32, tag="ps", name="ps_t")
                # filler matmuls: keep the tensor engine p-state ramped while
                # it waits for exp to free the ring; overwritten by start=True
                for f in range(FILL):
                    nc.tensor.matmul(
                        cur["ps"][:, 2 * f : 2 * f + 2, :],
                        lhsT=ident_sb, rhs=masks_sb[:, :, :],
                        start=True, stop=True,
                    )
            g = cur["n"]
            ps_t = cur["ps"]
            masked = j == i
            nc.tensor.matmul(
                ps_t[:, g, :], lhsT=kT_sb[:, r, j, :], rhs=qT_sb[:, i, :],
                start=True, stop=not masked,
            )
            if masked:
                nc.tensor.matmul(
                    ps_t[:, g, :], lhsT=ident_sb, rhs=masks_sb[:, r, :],
                    start=False, stop=True,
                )
            cur["meta"].append((i, r, j))
            cur["n"] += 1
            if cur["n"] == GPT:
                flush_tile()

        def wave(w):
            """Attention for my q blocks 2w, 2w+1 (k slots j <= i ready)."""
            get_acc(2 * w)
            for i in (2 * w, 2 * w + 1):
                for j in range(i + 1):
                    for r in range(2):
                        group(i, r, j)

        # ---- pipelined emission ----
        x_dma(0)
        x_dma(1)

        # PE warmup: dependency-free matmuls so the tensor engine is ramped
        # when proj 0's data lands
        wps = ps.tile([128, GPT, 128], F32, tag="ps", name="wps")
        for wi in range(24):
            nc.tensor.matmul(
                wps[:, wi % GPT, :], lhsT=ident_sb, rhs=ident_sb,
                start=True, stop=True,
            )

        # readback(p) trails exchange_cc(p) by one piece so its wait on the
        # collective resolves before it reaches the sync queue head; wave(w)
        # sits between readback(w) and readback(w+1) (tile-granular deps).
        proj_piece(0)
        exchange_cc(0)
        x_dma(2)
        proj_piece(1)
        exchange_cc(1)
        readback(0)
        x_dma(3)
        proj_piece(2)
        exchange_cc(2)
        readback(1)
        wave(0)
        flush_tile()  # release wave 0 without waiting on wave 1 data
        x_dma(4)
        proj_piece(3)
        exchange_cc(3)
        readback(2)
        wave(1)
        x_dma(5)
        proj_piece(4)
        exchange_cc(4)
        readback(3)
        wave(2)
        x_dma(6)
        proj_piece(5)
        exchange_cc(5)
        readback(4)
        wave(3)
        x_dma(7)
        proj_piece(6)
        exchange_cc(6)
        readback(5)
        wave(4)
        proj_piece(7)
        exchange_cc(7)
        readback(6)
        wave(5)
        readback(7)
        wave(6)
        wave(7)
        flush_tile()
        while deferred:
            emit_out(deferred.pop(0))

    nc.finalize()
    return nc


_PROGRAM_CACHE = {}


def _get_program():
    if "prog" not in _PROGRAM_CACHE:
        _PROGRAM_CACHE["prog"] = build_program()
    return _PROGRAM_CACHE["prog"]


def _bf16(a):
    import ml_dtypes
    return np.asarray(a).astype(ml_dtypes.bfloat16)


def _make_masks(parity: int) -> np.ndarray:
    """[2, 128, 128] additive mask tiles for the j == i k slots, per rank.

    q block i is natural block n = 2i + parity; rank-r slot i is natural
    block 2i + r.  r == parity -> the diagonal block (strict lower triangle
    of scores^T masked: k row p > q col j).  Otherwise rank 0's slot i
    (natural 2i) is fully valid for parity 1, and rank 1's slot i (natural
    2i+1) is fully masked for parity 0.
    """
    p = np.arange(128)[:, None]
    j = np.arange(128)[None, :]
    tri = np.where(p > j, MASK_VAL, 0.0).astype(np.float32)
    full = np.full((128, 128), MASK_VAL, np.float32)
    zero = np.zeros((128, 128), np.float32)
    if parity == 0:
        m = np.stack([tri, full])
    else:
        m = np.stack([zero, tri])
    return _bf16(m)


def kernel(x, Wq, bq, Wk, bk, Wv, bv):
    x = np.asarray(x, dtype=np.float32)
    wall = np.zeros((D, 256), np.float32)
    wall[:, 0:H] = np.asarray(Wq)
    wall[:, H : 2 * H] = np.asarray(Wk)
    wall[:, 2 * H : 3 * H] = np.asarray(Wv)
    wall = _bf16(wall)
    bqk = np.concatenate(
        [np.asarray(bq), np.asarray(bk)]
    ).astype(np.float32).reshape(2 * H, 1)
    bv_ = np.asarray(bv, dtype=np.float32).reshape(1, H)
    ident = _bf16(np.eye(128, dtype=np.float32))

    nc = _get_program()

    in_maps = []
    for core in range(NCORES):
        b, parity = core // 2, core % 2
        xTp = np.ascontiguousarray(
            x[b].T.reshape(D, 32, 128)[:, parity::2, :].reshape(D, S // 2)
        )
        in_maps.append(
            {
                "xT": _bf16(xTp),
                "wall": wall,
                "bqk": bqk,
                "bv": bv_,
                "masks": _make_masks(parity),
                "ident": ident,
            }
        )

    res = run_bass_kernel_spmd(nc, in_maps, list(range(NCORES)))

    out = np.empty((B, S, H), np.float32)
    for core in range(NCORES):
        b, parity = core // 2, core % 2
        o = np.asarray(res.results[core]["out"], np.float32).reshape(NMYB, 128, H)
        out[b].reshape(32, 128, H)[parity::2] = o
    return out


# revision 18
# speedup vs baseline: 1.1215x; 1.1215x over previous
"""Causal single-head attention (B=4, S=4096, D=1024, H=64) on 8 TRN2 NeuronCores.

Strategy (v3: sequence-parallel, peer k/v computed locally)
-----------------------------------------------------------
Data-parallel over batch (2 cores per batch element); within a pair the q rows
are split by 128-row block parity (even core: natural blocks 0,2,..,30; odd:
1,3,..,31), which load-balances the causal triangle.  Each core loads BOTH
halves of its batch element's activations (bf16, 8 MB) and computes q for its
own blocks plus k/v for ALL 32 blocks locally — no collectives, no exchange:
every dependency in the attention pipeline is a short on-core PE->DVE->PE hop.
Each core then produces COMPLETE attention rows for its own q blocks, so the
softmax denominator is a per-partition scalar and outputs are written directly.

Layouts chosen for the TRN2 cost model (matmul cost = moving columns only):
- All matmuls bf16 (1 col/cycle at any width; f32 weights would cost 4x).
- Projections: my tokens stream through lhsT=[Wq|Wk] (128 wide); peer tokens
  stream through lhsT=[Wk|Wv] — a contiguous slice of the same packed weight
  wall.  My v is projected directly in natural [token, h] layout (x chunk as
  lhsT, Wv moving: 64 cols per block); peer v arrives transposed in the
  [Wk|Wv] pass and is turned natural by a PE transpose (64 cols each).
- kT is [h, kpos] so scores^T tiles [128 k, 128 q] come out k-on-partitions,
  which is exactly the lhsT layout the out matmul wants:
  out_acc[128 q, 65] += lhsT=exp_tile, rhs=v_nat[128 k, 65] — only 65 moving
  cols per (k block, q block).  Col 64 of v_nat is constant 1, so the softmax
  denominator accumulates as output column 64 for free.
- The causal mask is applied ON the tensor engine: an extra accumulating
  matmul lhsT=I, rhs=mask_tile adds -8e9 into the banded scores PSUM.
  Host-computed mask data absorbs the even/odd parity asymmetry (the SPMD
  instruction stream is identical on all 8 cores): slot (0, j) holds my
  parity's block j, slot (1, j) the peer's; for q block i, slot (0, i) is
  always the diagonal (triangular mask) and slot (1, i) is fully masked on
  even cores / fully valid on odd cores, so extents are uniformly 2i+2.
- exp runs on the scalar engine out of PSUM in [128, 8*128] tiles (35 ops
  total) with the 1/8 scale folded in; masked entries underflow to exactly 0,
  matching the reference's -1e9 semantics.  exp outputs bf16; the scalar
  queue carries ONLY exp so it never sits behind a waiting DMA.
- Normalization: out_acc col 64 is the denominator (per-partition scalar) ->
  DVE reciprocal + tensor_scalar_mul, then direct DMA of the final rows.

The host only does layout work plus the fp32->bf16 cast of the inputs
(~5e-3 worst-case relative error, well inside the 2e-2 gate).
"""

import numpy as np
from contextlib import ExitStack

import concourse.bass as bass
import concourse.mybir as mybir
import concourse.tile as tile
from concourse import bacc
from concourse.bass_utils import run_bass_kernel_spmd
from concourse.masks import make_identity

F32 = mybir.dt.float32
BF16 = mybir.dt.bfloat16

B, S, D, H = 4, 4096, 1024, 64
NCORES = 8
NCH = D // 128        # contraction chunks
NMYB = 16             # my q blocks per core
MASK_VAL = -8.0e9     # raw-score mask; exp(0.125*(s+MASK_VAL)) == 0
SCALE = 0.125         # 1/sqrt(H)
GPT = 8               # score/exp groups per PSUM tile ([128, GPT*128])


def build_program(with_cc: bool = True):
    nc = bacc.Bacc(num_devices=NCORES)

    xT = nc.declare_dram_parameter("xT", [D, S], BF16, isOutput=False)
    wall = nc.declare_dram_parameter("wall", [D, 256], BF16, isOutput=False)
    bqk = nc.declare_dram_parameter("bqk", [2 * H, 1], F32, isOutput=False)
    bv = nc.declare_dram_parameter("bv", [1, H], F32, isOutput=False)
    masks = nc.declare_dram_parameter("masks", [2, 128, 128], BF16, isOutput=False)
    ident = nc.declare_dram_parameter("ident", [128, 128], BF16, isOutput=False)
    out = nc.declare_dram_parameter("out", [S // 2, H], F32, isOutput=True)

    xT3 = xT.rearrange("(c p) s -> p c s", p=128)        # [128, 8, 4096]
    wall3 = wall.rearrange("(c p) h -> p c h", p=128)    # [128, 8, 256]
    masks3 = masks.rearrange("m p j -> p m j")           # [128, 2, 128]
    out3 = out.rearrange("(n p) h -> p n h", p=128)      # [128, 16, 64]

    with ExitStack() as ctx:
        tc = ctx.enter_context(tile.TileContext(nc))

        singles = ctx.enter_context(tc.tile_pool(name="singles", bufs=1))

        xt_all = singles.tile([128, NCH, S], BF16)
        wall_sb = singles.tile([128, NCH, 256], BF16)
        bqk_sb = singles.tile([2 * H, 1], F32)
        bv_bc = singles.tile([128, H], F32)
        masks_sb = singles.tile([128, 2, 128], BF16)
        ident_sb = singles.tile([128, 128], BF16)
        ident64 = singles.tile([H, H], F32)
        make_identity(nc, ident64)

        # priority order on the sync queue: tiny params, weights, then the x
        # stream (my/peer piece pairs in consumption order)
        nc.sync.dma_start(out=ident_sb, in_=ident[:, :])
        nc.sync.dma_start(out=bqk_sb, in_=bqk[:, :])
        nc.sync.dma_start(out=masks_sb, in_=masks3)
        bv_b = bass.AP(tensor=bv[:, :].tensor, offset=bv[:, :].offset,
                       ap=[[0, 128], [1, H]])
        nc.sync.dma_start(out=bv_bc, in_=bv_b)
        nc.sync.dma_start(out=wall_sb, in_=wall3)

        def x_dma(lo, n):
            nc.sync.dma_start(
                out=xt_all[:, :, lo : lo + n], in_=xT3[:, :, lo : lo + n]
            )

        # persistent SBUF state; slot (0, j) = my block j, (1, j) = peer's
        qT_sb = singles.tile([H, NMYB, 128], BF16)
        kT_sb = singles.tile([H, 2, NMYB, 128], BF16)
        v_sb = singles.tile([128, 2, NMYB, 65], BF16)
        out_stage = singles.tile([128, NMYB, H], F32)
        rcp_sb = singles.tile([128, NMYB], F32)
        nc.vector.memset(v_sb[:, :, :, H : H + 1], 1.0)

        pj = ctx.enter_context(tc.tile_pool(name="pj", bufs=2, space="PSUM"))
        ps = ctx.enter_context(tc.tile_pool(name="ps", bufs=2, space="PSUM"))
        pacc = ctx.enter_context(tc.tile_pool(name="pacc", bufs=2, space="PSUM"))
        pexp_pool = ctx.enter_context(tc.tile_pool(name="pexp", bufs=4))
        vpt = ctx.enter_context(tc.tile_pool(name="vpt", bufs=2))

        def proj_mine(p):
            """q/k/v for my token blocks 2p, 2p+1 (xt cols 256p..)."""
            lo = 256 * p
            b0 = 2 * p
            # [q|k] 0:256 | v_nat b0 256:320 | v_nat b0+1 320:384
            pjt = pj.tile([128, 384], F32, tag="pj", name="pjt")
            for ch in range(NCH):
                nc.tensor.matmul(
                    pjt[:, 0:256], lhsT=wall_sb[:, ch, 0 : 2 * H],
                    rhs=xt_all[:, ch, lo : lo + 256],
                    start=(ch == 0), stop=(ch == NCH - 1),
                )
            for b2 in range(2):
                for ch in range(NCH):
                    nc.tensor.matmul(
                        pjt[:, 256 + 64 * b2 : 320 + 64 * b2],
                        lhsT=xt_all[:, ch, lo + 128 * b2 : lo + 128 * b2 + 128],
                        rhs=wall_sb[:, ch, 2 * H : 3 * H],
                        start=(ch == 0), stop=(ch == NCH - 1),
                    )
            nc.vector.tensor_scalar_add(
                qT_sb[:, b0 : b0 + 2, :], pjt[0:H, 0:256], bqk_sb[0:H, :]
            )
            nc.vector.tensor_scalar_add(
                kT_sb[:, 0, b0 : b0 + 2, :], pjt[H : 2 * H, 0:256],
                bqk_sb[H : 2 * H, :],
            )
            nc.vector.tensor_add(v_sb[:, 0, b0, 0:H], pjt[:, 256:320], bv_bc)
            nc.vector.tensor_add(v_sb[:, 0, b0 + 1, 0:H], pjt[:, 320:384], bv_bc)

        def proj_peer(p):
            """k/vT for peer token blocks 2p, 2p+1 (xt cols 2048+256p..)."""
            lo = S // 2 + 256 * p
            b0 = 2 * p
            # [k|vT] 0:256 | v_nat b0 256:320 | v_nat b0+1 320:384
            pjt = pj.tile([128, 384], F32, tag="pj", name="pjt")
            for ch in range(NCH):
                nc.tensor.matmul(
                    pjt[:, 0:256], lhsT=wall_sb[:, ch, H : 3 * H],
                    rhs=xt_all[:, ch, lo : lo + 256],
                    start=(ch == 0), stop=(ch == NCH - 1),
                )
            vpt_t = vpt.tile([H, 2, 128], F32, tag="vpt", name="vpt_t")
            nc.vector.tensor_scalar_add(
                kT_sb[:, 1, b0 : b0 + 2, :], pjt[0:H, 0:256],
                bqk_sb[H : 2 * H, :],
            )
            nc.vector.tensor_copy(vpt_t, pjt[H : 2 * H, 0:256])
            for b2 in range(2):
                nc.tensor.transpose(
                    pjt[:, 256 + 64 * b2 : 320 + 64 * b2], vpt_t[:, b2, :],
                    ident64,
                )
                nc.vector.tensor_add(
                    v_sb[:, 1, b0 + b2, 0:H],
                    pjt[:, 256 + 64 * b2 : 320 + 64 * b2], bv_bc,
                )

        # ---- attention pipeline ----
        cur = {"ps": None, "n": 0, "meta": []}
        deferred = []  # (ps_tile, pexp_tile, meta) awaiting out-matmul emission
        acc_of = {}

        def get_acc(i):
            if i not in acc_of:
                t = pacc.tile([128, 2, 66], F32, tag="pacc", name="acc_t")
                acc_of[i] = t
                acc_of[i + 1] = t
            return acc_of[i]

        def emit_out(batch):
            """Out matmuls (and norms) for a completed exp tile."""
            ps_t, px_t, meta = batch
            for g, (i, r, j) in enumerate(meta):
                nc.tensor.matmul(
                    acc_of[i][:, i % 2, 0:65],
                    lhsT=px_t[:, g, :],
                    rhs=v_sb[:, r, j, :],
                    start=(r == 0 and j == 0),
                    stop=(r == 1 and j == i),
                )
                if r == 1 and j == i:
                    nc.vector.reciprocal(
                        rcp_sb[:, i : i + 1], acc_of[i][:, i % 2, 64:65]
                    )
                    nc.vector.tensor_scalar_mul(
                        out_stage[:, i, :], acc_of[i][:, i % 2, 0:64],
                        rcp_sb[:, i : i + 1],
                    )
                    if i % 4 == 3:
                        g4 = i // 4
                        nc.gpsimd.dma_start(
                            out=out3[:, 4 * g4 : 4 * g4 + 4, :],
                            in_=out_stage[:, 4 * g4 : 4 * g4 + 4, :],
                        )

        def flush_tile():
            """Close the current scores tile: exp it, queue its out matmuls."""
            if cur["n"] == 0:
                return
            ps_t, meta = cur["ps"], cur["meta"]
            px_t = pexp_pool.tile([128, GPT, 128], BF16, tag="px", name="px_t")
            n = cur["n"]
            nc.scalar.activation(
                px_t[:, 0:n, :], ps_t[:, 0:n, :],
                mybir.ActivationFunctionType.Exp, scale=SCALE,
            )
            deferred.append((ps_t, px_t, list(meta)))
            cur["ps"], cur["n"], cur["meta"] = None, 0, []
            if len(deferred) > 2:
                emit_out(deferred.pop(0))

        FILL = 2

        def group(i, r, j):
            """Scores (+mask) for q block i vs k slot (r, j)."""
            if cur["ps"] is None:
                cur["ps"] = ps.tile([128, GPT, 128], F32, tag="ps", name="ps_t")
                # filler matmuls: keep the tensor engine p-state ramped while
                # it waits for exp to free the ring; overwritten by start=True
                for f in range(FILL):
                    nc.tensor.matmul(
                        cur["ps"][:, 2 * f : 2 * f + 2, :],
                        lhsT=ident_sb, rhs=masks_sb[:, :, :],
                        start=True, stop=True,
                    )
            g = cur["n"]
            ps_t = cur["ps"]
            masked = j == i
            nc.tensor.matmul(
                ps_t[:, g, :], lhsT=kT_sb[:, r, j, :], rhs=qT_sb[:, i, :],
                start=True, stop=not masked,
            )
            if masked:
                nc.tensor.matmul(
                    ps_t[:, g, :], lhsT=ident_sb, rhs=masks_sb[:, r, :],
                    start=False, stop=True,
                )
            cur["meta"].append((i, r, j))
            cur["n"] += 1
            if cur["n"] == GPT:
                flush_tile()

        def wave(w):
            """Attention for my q blocks 2w, 2w+1 (k slots j <= i ready)."""
            get_acc(2 * w)
            for i in (2 * w, 2 * w + 1):
                for j in range(i + 1):
                    for r in range(2):
                        group(i, r, j)

        # ---- pipelined emission ----
        x_dma(0, 256)           # my blocks 0-1
        x_dma(S // 2, 256)      # peer blocks 0-1

        # PE warmup: dependency-free matmuls so the tensor engine is at full
        # p-state when proj 0's data lands
        wps = ps.tile([128, GPT, 128], F32, tag="ps", name="wps")
        for wi in range(24):
            nc.tensor.matmul(
                wps[:, wi % GPT, :], lhsT=ident_sb, rhs=ident_sb,
                start=True, stop=True,
            )

        for p in range(8):
            proj_mine(p)
            proj_peer(p)
            if p < 7:
                x_dma(256 * (p + 1), 256)
                x_dma(S // 2 + 256 * (p + 1), 256)
            if p >= 1:
                wave(p - 1)
                if p == 1:
                    flush_tile()  # release wave 0 without waiting on wave 1
        wave(6)
        wave(7)
        flush_tile()
        while deferred:
            emit_out(deferred.pop(0))

    nc.finalize()
    return nc


_PROGRAM_CACHE = {}


def _get_program():
    if "prog" not in _PROGRAM_CACHE:
        _PROGRAM_CACHE["prog"] = build_program()
    return _PROGRAM_CACHE["prog"]


def _bf16(a):
    import ml_dtypes
    return np.asarray(a).astype(ml_dtypes.bfloat16)


def _make_masks(parity: int) -> np.ndarray:
    """[2, 128, 128] additive mask tiles for the j == i k slot.

    Slot (0, i) is my own block i = the diagonal (strict lower triangle of
    scores^T masked: k row p > q col j).  Slot (1, i) is the peer's block i:
    natural 2i+1 > 2i for even cores (fully masked), natural 2i < 2i+1 for
    odd cores (fully valid).
    """
    p = np.arange(128)[:, None]
    j = np.arange(128)[None, :]
    tri = np.where(p > j, MASK_VAL, 0.0).astype(np.float32)
    full = np.full((128, 128), MASK_VAL, np.float32)
    zero = np.zeros((128, 128), np.float32)
    m = np.stack([tri, full if parity == 0 else zero])
    return _bf16(m)


def kernel(x, Wq, bq, Wk, bk, Wv, bv):
    x = np.asarray(x, dtype=np.float32)
    wall = np.zeros((D, 256), np.float32)
    wall[:, 0:H] = np.asarray(Wq)
    wall[:, H : 2 * H] = np.asarray(Wk)
    wall[:, 2 * H : 3 * H] = np.asarray(Wv)
    wall = _bf16(wall)
    bqk = np.concatenate(
        [np.asarray(bq), np.asarray(bk)]
    ).astype(np.float32).reshape(2 * H, 1)
    bv_ = np.asarray(bv, dtype=np.float32).reshape(1, H)
    ident = _bf16(np.eye(128, dtype=np.float32))

    nc = _get_program()

    in_maps = []
    for core in range(NCORES):
        b, parity = core // 2, core % 2
        xb = x[b].T.reshape(D, 32, 128)
        mine = xb[:, parity::2, :].reshape(D, S // 2)
        peer = xb[:, 1 - parity :: 2, :].reshape(D, S // 2)
        xTp = np.ascontiguousarray(np.concatenate([mine, peer], axis=1))
        in_maps.append(
            {
                "xT": _bf16(xTp),
                "wall": wall,
                "bqk": bqk,
                "bv": bv_,
                "masks": _make_masks(parity),
                "ident": ident,
            }
        )

    res = run_bass_kernel_spmd(nc, in_maps, list(range(NCORES)))

    out = np.empty((B, S, H), np.float32)
    for core in range(NCORES):
        b, parity = core // 2, core % 2
        o = np.asarray(res.results[core]["out"], np.float32).reshape(NMYB, 128, H)
        out[b].reshape(32, 128, H)[parity::2] = o
    return out


# revision 19
# speedup vs baseline: 1.2080x; 1.0772x over previous
"""Causal single-head attention (B=4, S=4096, D=1024, H=64) on 8 TRN2 NeuronCores.

Strategy (v3: sequence-parallel, peer k/v computed locally)
-----------------------------------------------------------
Data-parallel over batch (2 cores per batch element); within a pair the q rows
are split by 128-row block parity (even core: natural blocks 0,2,..,30; odd:
1,3,..,31), which load-balances the causal triangle.  Each core loads BOTH
halves of its batch element's activations (bf16, 8 MB) and computes q for its
own blocks plus k/v for ALL 32 blocks locally — no collectives, no exchange:
every dependency in the attention pipeline is a short on-core PE->DVE->PE hop.
Each core then produces COMPLETE attention rows for its own q blocks, so the
softmax denominator is a per-partition scalar and outputs are written directly.

Layouts chosen for the TRN2 cost model (matmul cost = moving columns only):
- All matmuls bf16 (1 col/cycle at any width; f32 weights would cost 4x).
- Projections: my tokens stream through lhsT=[Wq|Wk] (128 wide); peer tokens
  stream through lhsT=[Wk|Wv] — a contiguous slice of the same packed weight
  wall.  My v is projected directly in natural [token, h] layout (x chunk as
  lhsT, Wv moving: 64 cols per block); peer v arrives transposed in the
  [Wk|Wv] pass and is turned natural by a PE transpose (64 cols each).
- kT is [h, kpos] so scores^T tiles [128 k, 128 q] come out k-on-partitions,
  which is exactly the lhsT layout the out matmul wants:
  out_acc[128 q, 65] += lhsT=exp_tile, rhs=v_nat[128 k, 65] — only 65 moving
  cols per (k block, q block).  Col 64 of v_nat is constant 1, so the softmax
  denominator accumulates as output column 64 for free.
- The causal mask is applied ON the tensor engine: an extra accumulating
  matmul lhsT=I, rhs=mask_tile adds -8e9 into the banded scores PSUM.
  Host-computed mask data absorbs the even/odd parity asymmetry (the SPMD
  instruction stream is identical on all 8 cores): slot (0, j) holds my
  parity's block j, slot (1, j) the peer's; for q block i, slot (0, i) is
  always the diagonal (triangular mask) and slot (1, i) is fully masked on
  even cores / fully valid on odd cores, so extents are uniformly 2i+2.
- exp runs on the scalar engine out of PSUM in [128, 8*128] tiles (35 ops
  total) with the 1/8 scale folded in; masked entries underflow to exactly 0,
  matching the reference's -1e9 semantics.  exp outputs bf16; the scalar
  queue carries ONLY exp so it never sits behind a waiting DMA.
- Normalization: out_acc col 64 is the denominator (per-partition scalar) ->
  DVE reciprocal + tensor_scalar_mul, then direct DMA of the final rows.

The host only does layout work plus the fp32->bf16 cast of the inputs
(~5e-3 worst-case relative error, well inside the 2e-2 gate).
"""

import numpy as np
from contextlib import ExitStack

import concourse.bass as bass
import concourse.mybir as mybir
import concourse.tile as tile
from concourse import bacc
from concourse.bass_utils import run_bass_kernel_spmd
from concourse.masks import make_identity

F32 = mybir.dt.float32
BF16 = mybir.dt.bfloat16

B, S, D, H = 4, 4096, 1024, 64
NCORES = 8
NCH = D // 128        # contraction chunks
NMYB = 16             # my q blocks per core
MASK_VAL = -8.0e9     # raw-score mask; exp(0.125*(s+MASK_VAL)) == 0
SCALE = 0.125         # 1/sqrt(H)
GPT = 8               # score/exp groups per PSUM tile ([128, GPT*128])


def build_program(with_cc: bool = True):
    nc = bacc.Bacc(num_devices=NCORES)

    xT = nc.declare_dram_parameter("xT", [D, S], BF16, isOutput=False)
    wall = nc.declare_dram_parameter("wall", [D, 256], BF16, isOutput=False)
    bqk = nc.declare_dram_parameter("bqk", [2 * H, 1], F32, isOutput=False)
    bv = nc.declare_dram_parameter("bv", [1, H], F32, isOutput=False)
    masks = nc.declare_dram_parameter("masks", [2, 128, 128], BF16, isOutput=False)
    ident = nc.declare_dram_parameter("ident", [128, 128], BF16, isOutput=False)
    out = nc.declare_dram_parameter("out", [S // 2, H], F32, isOutput=True)

    xT3 = xT.rearrange("(c p) s -> p c s", p=128)        # [128, 8, 4096]
    wall3 = wall.rearrange("(c p) h -> p c h", p=128)    # [128, 8, 256]
    masks3 = masks.rearrange("m p j -> p m j")           # [128, 2, 128]
    out3 = out.rearrange("(n p) h -> p n h", p=128)      # [128, 16, 64]

    with ExitStack() as ctx:
        tc = ctx.enter_context(tile.TileContext(nc))

        singles = ctx.enter_context(tc.tile_pool(name="singles", bufs=1))

        xt_all = singles.tile([128, NCH, S], BF16)
        wall_sb = singles.tile([128, NCH, 256], BF16)
        bqk_sb = singles.tile([2 * H, 1], F32)
        bv_bc = singles.tile([128, H], F32)
        masks_sb = singles.tile([128, 2, 128], BF16)
        ident_sb = singles.tile([128, 128], BF16)
        ident64 = singles.tile([H, H], F32)
        make_identity(nc, ident64)

        # priority order on the sync queue: tiny params, weights, then the x
        # stream (my/peer piece pairs in consumption order)
        nc.sync.dma_start(out=ident_sb, in_=ident[:, :])
        nc.sync.dma_start(out=bqk_sb, in_=bqk[:, :])
        nc.sync.dma_start(out=masks_sb, in_=masks3)
        bv_b = bass.AP(tensor=bv[:, :].tensor, offset=bv[:, :].offset,
                       ap=[[0, 128], [1, H]])
        nc.sync.dma_start(out=bv_bc, in_=bv_b)
        nc.sync.dma_start(out=wall_sb, in_=wall3)

        def x_dma(lo, n):
            nc.sync.dma_start(
                out=xt_all[:, :, lo : lo + n], in_=xT3[:, :, lo : lo + n]
            )

        # persistent SBUF state; slot (0, j) = my block j, (1, j) = peer's
        qT_sb = singles.tile([H, NMYB, 128], BF16)
        kT_sb = singles.tile([H, 2, NMYB, 128], BF16)
        v_sb = singles.tile([128, 2, NMYB, 65], BF16)
        out_stage = singles.tile([128, NMYB, H], F32)
        rcp_sb = singles.tile([128, NMYB], F32)
        nc.vector.memset(v_sb[:, :, :, H : H + 1], 1.0)

        pj = ctx.enter_context(tc.tile_pool(name="pj", bufs=2, space="PSUM"))
        ps = ctx.enter_context(tc.tile_pool(name="ps", bufs=2, space="PSUM"))
        pacc = ctx.enter_context(tc.tile_pool(name="pacc", bufs=2, space="PSUM"))
        pexp_pool = ctx.enter_context(tc.tile_pool(name="pexp", bufs=4))
        vpt = ctx.enter_context(tc.tile_pool(name="vpt", bufs=2))

        def proj_mine(p):
            """q/k/v for my token blocks 2p, 2p+1 (xt cols 256p..)."""
            lo = 256 * p
            b0 = 2 * p
            # [q|k] 0:256 | v_nat b0 256:320 | v_nat b0+1 320:384
            pjt = pj.tile([128, 384], F32, tag="pj", name="pjt")
            for ch in range(NCH):
                nc.tensor.matmul(
                    pjt[:, 0:256], lhsT=wall_sb[:, ch, 0 : 2 * H],
                    rhs=xt_all[:, ch, lo : lo + 256],
                    start=(ch == 0), stop=(ch == NCH - 1),
                )
            for b2 in range(2):
                for ch in range(NCH):
                    nc.tensor.matmul(
                        pjt[:, 256 + 64 * b2 : 320 + 64 * b2],
                        lhsT=xt_all[:, ch, lo + 128 * b2 : lo + 128 * b2 + 128],
                        rhs=wall_sb[:, ch, 2 * H : 3 * H],
                        start=(ch == 0), stop=(ch == NCH - 1),
                    )
            nc.vector.tensor_scalar_add(
                qT_sb[:, b0 : b0 + 2, :], pjt[0:H, 0:256], bqk_sb[0:H, :]
            )
            nc.vector.tensor_scalar_add(
                kT_sb[:, 0, b0 : b0 + 2, :], pjt[H : 2 * H, 0:256],
                bqk_sb[H : 2 * H, :],
            )
            nc.vector.tensor_add(v_sb[:, 0, b0, 0:H], pjt[:, 256:320], bv_bc)
            nc.vector.tensor_add(v_sb[:, 0, b0 + 1, 0:H], pjt[:, 320:384], bv_bc)

        def proj_peer(p):
            """k/vT for peer token blocks 2p, 2p+1 (xt cols 2048+256p..)."""
            lo = S // 2 + 256 * p
            b0 = 2 * p
            # [k|vT] 0:256 | v_nat b0 256:320 | v_nat b0+1 320:384
            pjt = pj.tile([128, 384], F32, tag="pj", name="pjt")
            for ch in range(NCH):
                nc.tensor.matmul(
                    pjt[:, 0:256], lhsT=wall_sb[:, ch, H : 3 * H],
                    rhs=xt_all[:, ch, lo : lo + 256],
                    start=(ch == 0), stop=(ch == NCH - 1),
                )
            vpt_t = vpt.tile([H, 2, 128], F32, tag="vpt", name="vpt_t")
            nc.vector.tensor_scalar_add(
                kT_sb[:, 1, b0 : b0 + 2, :], pjt[0:H, 0:256],
                bqk_sb[H : 2 * H, :],
            )
            nc.vector.tensor_copy(vpt_t, pjt[H : 2 * H, 0:256])
            for b2 in range(2):
                nc.tensor.transpose(
                    pjt[:, 256 + 64 * b2 : 320 + 64 * b2], vpt_t[:, b2, :],
                    ident64,
                )
                nc.vector.tensor_add(
                    v_sb[:, 1, b0 + b2, 0:H],
                    pjt[:, 256 + 64 * b2 : 320 + 64 * b2], bv_bc,
                )

        # ---- attention pipeline ----
        cur = {"ps": None, "n": 0, "meta": []}
        deferred = []  # (ps_tile, pexp_tile, meta) awaiting out-matmul emission
        acc_of = {}

        def get_acc(i):
            if i not in acc_of:
                t = pacc.tile([128, 2, 66], F32, tag="pacc", name="acc_t")
                acc_of[i] = t
                acc_of[i + 1] = t
            return acc_of[i]

        def emit_out(batch):
            """Out matmuls (and norms) for a completed exp tile."""
            ps_t, px_t, meta = batch
            for g, (i, r, j) in enumerate(meta):
                nc.tensor.matmul(
                    acc_of[i][:, i % 2, 0:65],
                    lhsT=px_t[:, g, :],
                    rhs=v_sb[:, r, j, :],
                    start=(r == 0 and j == 0),
                    stop=(r == 1 and j == i),
                )
                if r == 1 and j == i:
                    nc.vector.reciprocal(
                        rcp_sb[:, i : i + 1], acc_of[i][:, i % 2, 64:65]
                    )
                    nc.vector.tensor_scalar_mul(
                        out_stage[:, i, :], acc_of[i][:, i % 2, 0:64],
                        rcp_sb[:, i : i + 1],
                    )
                    if i % 4 == 3:
                        g4 = i // 4
                        nc.gpsimd.dma_start(
                            out=out3[:, 4 * g4 : 4 * g4 + 4, :],
                            in_=out_stage[:, 4 * g4 : 4 * g4 + 4, :],
                        )

        def flush_tile():
            """Close the current scores tile: exp it, queue its out matmuls."""
            if cur["n"] == 0:
                return
            ps_t, meta = cur["ps"], cur["meta"]
            px_t = pexp_pool.tile([128, GPT, 128], BF16, tag="px", name="px_t")
            n = cur["n"]
            nc.scalar.activation(
                px_t[:, 0:n, :], ps_t[:, 0:n, :],
                mybir.ActivationFunctionType.Exp, scale=SCALE,
            )
            deferred.append((ps_t, px_t, list(meta)))
            cur["ps"], cur["n"], cur["meta"] = None, 0, []
            if len(deferred) > 2:
                emit_out(deferred.pop(0))

        FILL = 0

        def group(i, r, j):
            """Scores (+mask) for q block i vs k slot (r, j)."""
            if cur["ps"] is None:
                cur["ps"] = ps.tile([128, GPT, 128], F32, tag="ps", name="ps_t")
                # filler matmuls: keep the tensor engine p-state ramped while
                # it waits for exp to free the ring; overwritten by start=True
                for f in range(FILL):
                    nc.tensor.matmul(
                        cur["ps"][:, 2 * f : 2 * f + 2, :],
                        lhsT=ident_sb, rhs=masks_sb[:, :, :],
                        start=True, stop=True,
                    )
            g = cur["n"]
            ps_t = cur["ps"]
            masked = j == i
            nc.tensor.matmul(
                ps_t[:, g, :], lhsT=kT_sb[:, r, j, :], rhs=qT_sb[:, i, :],
                start=True, stop=not masked,
            )
            if masked:
                nc.tensor.matmul(
                    ps_t[:, g, :], lhsT=ident_sb, rhs=masks_sb[:, r, :],
                    start=False, stop=True,
                )
            cur["meta"].append((i, r, j))
            cur["n"] += 1
            if cur["n"] == GPT:
                flush_tile()

        def wave(w):
            """Attention for my q blocks 2w, 2w+1 (k slots j <= i ready)."""
            get_acc(2 * w)
            for i in (2 * w, 2 * w + 1):
                for j in range(i + 1):
                    for r in range(2):
                        group(i, r, j)

        # ---- pipelined emission ----
        x_dma(0, 256)           # my blocks 0-1
        x_dma(S // 2, 256)      # peer blocks 0-1

        # PE warmup: dependency-free matmuls so the tensor engine is at full
        # p-state when proj 0's data lands
        wps = ps.tile([128, GPT, 128], F32, tag="ps", name="wps")
        for wi in range(24):
            nc.tensor.matmul(
                wps[:, wi % GPT, :], lhsT=ident_sb, rhs=ident_sb,
                start=True, stop=True,
            )

        for p in range(8):
            proj_mine(p)
            proj_peer(p)
            if p < 7:
                x_dma(256 * (p + 1), 256)
                x_dma(S // 2 + 256 * (p + 1), 256)
            if p >= 1:
                wave(p - 1)
                if p == 1:
                    flush_tile()  # release wave 0 without waiting on wave 1
        wave(6)
        wave(7)
        flush_tile()
        while deferred:
            emit_out(deferred.pop(0))

    nc.finalize()
    return nc


_PROGRAM_CACHE = {}


def _get_program():
    if "prog" not in _PROGRAM_CACHE:
        _PROGRAM_CACHE["prog"] = build_program()
    return _PROGRAM_CACHE["prog"]


def _bf16(a):
    import ml_dtypes
    return np.asarray(a).astype(ml_dtypes.bfloat16)


def _make_masks(parity: int) -> np.ndarray:
    """[2, 128, 128] additive mask tiles for the j == i k slot.

    Slot (0, i) is my own block i = the diagonal (strict lower triangle of
    scores^T masked: k row p > q col j).  Slot (1, i) is the peer's block i:
    natural 2i+1 > 2i for even cores (fully masked), natural 2i < 2i+1 for
    odd cores (fully valid).
    """
    p = np.arange(128)[:, None]
    j = np.arange(128)[None, :]
    tri = np.where(p > j, MASK_VAL, 0.0).astype(np.float32)
    full = np.full((128, 128), MASK_VAL, np.float32)
    zero = np.zeros((128, 128), np.float32)
    m = np.stack([tri, full if parity == 0 else zero])
    return _bf16(m)


def kernel(x, Wq, bq, Wk, bk, Wv, bv):
    x = np.asarray(x, dtype=np.float32)
    wall = np.zeros((D, 256), np.float32)
    wall[:, 0:H] = np.asarray(Wq)
    wall[:, H : 2 * H] = np.asarray(Wk)
    wall[:, 2 * H : 3 * H] = np.asarray(Wv)
    wall = _bf16(wall)
    bqk = np.concatenate(
        [np.asarray(bq), np.asarray(bk)]
    ).astype(np.float32).reshape(2 * H, 1)
    bv_ = np.asarray(bv, dtype=np.float32).reshape(1, H)
    ident = _bf16(np.eye(128, dtype=np.float32))

    nc = _get_program()

    in_maps = []
    for core in range(NCORES):
        b, parity = core // 2, core % 2
        xb = x[b].T.reshape(D, 32, 128)
        mine = xb[:, parity::2, :].reshape(D, S // 2)
        peer = xb[:, 1 - parity :: 2, :].reshape(D, S // 2)
        xTp = np.ascontiguousarray(np.concatenate([mine, peer], axis=1))
        in_maps.append(
            {
                "xT": _bf16(xTp),
                "wall": wall,
                "bqk": bqk,
                "bv": bv_,
                "masks": _make_masks(parity),
                "ident": ident,
            }
        )

    res = run_bass_kernel_spmd(nc, in_maps, list(range(NCORES)))

    out = np.empty((B, S, H), np.float32)
    for core in range(NCORES):
        b, parity = core // 2, core % 2
        o = np.asarray(res.results[core]["out"], np.float32).reshape(NMYB, 128, H)
        out[b].reshape(32, 128, H)[parity::2] = o
    return out


# revision 20
# speedup vs baseline: 1.2097x; 1.0014x over previous
"""Causal single-head attention (B=4, S=4096, D=1024, H=64) on 8 TRN2 NeuronCores.

Strategy (v3: sequence-parallel, peer k/v computed locally)
-----------------------------------------------------------
Data-parallel over batch (2 cores per batch element); within a pair the q rows
are split by 128-row block parity (even core: natural blocks 0,2,..,30; odd:
1,3,..,31), which load-balances the causal triangle.  Each core loads BOTH
halves of its batch element's activations (bf16, 8 MB) and computes q for its
own blocks plus k/v for ALL 32 blocks locally — no collectives, no exchange:
every dependency in the attention pipeline is a short on-core PE->DVE->PE hop.
Each core then produces COMPLETE attention rows for its own q blocks, so the
softmax denominator is a per-partition scalar and outputs are written directly.

Layouts chosen for the TRN2 cost model (matmul cost = moving columns only):
- All matmuls bf16 (1 col/cycle at any width; f32 weights would cost 4x).
- Projections: my tokens stream through lhsT=[Wq|Wk] (128 wide); peer tokens
  stream through lhsT=[Wk|Wv] — a contiguous slice of the same packed weight
  wall.  My v is projected directly in natural [token, h] layout (x chunk as
  lhsT, Wv moving: 64 cols per block); peer v arrives transposed in the
  [Wk|Wv] pass and is turned natural by a PE transpose (64 cols each).
- kT is [h, kpos] so scores^T tiles [128 k, 128 q] come out k-on-partitions,
  which is exactly the lhsT layout the out matmul wants:
  out_acc[128 q, 65] += lhsT=exp_tile, rhs=v_nat[128 k, 65] — only 65 moving
  cols per (k block, q block).  Col 64 of v_nat is constant 1, so the softmax
  denominator accumulates as output column 64 for free.
- The causal mask is applied ON the tensor engine: an extra accumulating
  matmul lhsT=I, rhs=mask_tile adds -8e9 into the banded scores PSUM.
  Host-computed mask data absorbs the even/odd parity asymmetry (the SPMD
  instruction stream is identical on all 8 cores): slot (0, j) holds my
  parity's block j, slot (1, j) the peer's; for q block i, slot (0, i) is
  always the diagonal (triangular mask) and slot (1, i) is fully masked on
  even cores / fully valid on odd cores, so extents are uniformly 2i+2.
- exp runs on the scalar engine out of PSUM in [128, 8*128] tiles (35 ops
  total) with the 1/8 scale folded in; masked entries underflow to exactly 0,
  matching the reference's -1e9 semantics.  exp outputs bf16; the scalar
  queue carries ONLY exp so it never sits behind a waiting DMA.
- Normalization: out_acc col 64 is the denominator (per-partition scalar) ->
  DVE reciprocal + tensor_scalar_mul, then direct DMA of the final rows.

The host only does layout work plus the fp32->bf16 cast of the inputs
(~5e-3 worst-case relative error, well inside the 2e-2 gate).
"""

import numpy as np
from contextlib import ExitStack

import concourse.bass as bass
import concourse.mybir as mybir
import concourse.tile as tile
from concourse import bacc
from concourse.bass_utils import run_bass_kernel_spmd
from concourse.masks import make_identity

F32 = mybir.dt.float32
BF16 = mybir.dt.bfloat16

B, S, D, H = 4, 4096, 1024, 64
NCORES = 8
NCH = D // 128        # contraction chunks
NMYB = 16             # my q blocks per core
MASK_VAL = -8.0e9     # raw-score mask; exp(0.125*(s+MASK_VAL)) == 0
SCALE = 0.125         # 1/sqrt(H)
GPT = 8               # score/exp groups per PSUM tile ([128, GPT*128])


def build_program(with_cc: bool = True):
    nc = bacc.Bacc(num_devices=NCORES)

    xT = nc.declare_dram_parameter("xT", [D, S], BF16, isOutput=False)
    wall = nc.declare_dram_parameter("wall", [D, 256], BF16, isOutput=False)
    bqk = nc.declare_dram_parameter("bqk", [2 * H, 1], F32, isOutput=False)
    bv = nc.declare_dram_parameter("bv", [1, H], F32, isOutput=False)
    masks = nc.declare_dram_parameter("masks", [2, 128, 128], BF16, isOutput=False)
    ident = nc.declare_dram_parameter("ident", [128, 128], BF16, isOutput=False)
    out = nc.declare_dram_parameter("out", [S // 2, H], F32, isOutput=True)

    xT3 = xT.rearrange("(c p) s -> p c s", p=128)        # [128, 8, 4096]
    wall3 = wall.rearrange("(c p) h -> p c h", p=128)    # [128, 8, 256]
    masks3 = masks.rearrange("m p j -> p m j")           # [128, 2, 128]
    out3 = out.rearrange("(n p) h -> p n h", p=128)      # [128, 16, 64]

    with ExitStack() as ctx:
        tc = ctx.enter_context(tile.TileContext(nc))

        singles = ctx.enter_context(tc.tile_pool(name="singles", bufs=1))

        xt_all = singles.tile([128, NCH, S], BF16)
        wall_sb = singles.tile([128, NCH, 256], BF16)
        bqk_sb = singles.tile([2 * H, 1], F32)
        bv_bc = singles.tile([128, H], F32)
        masks_sb = singles.tile([128, 2, 128], BF16)
        ident_sb = singles.tile([128, 128], BF16)
        ident64 = singles.tile([H, H], F32)
        make_identity(nc, ident64)

        # priority order on the sync queue: tiny params, weights, then the x
        # stream (my/peer piece pairs in consumption order)
        nc.sync.dma_start(out=ident_sb, in_=ident[:, :])
        nc.sync.dma_start(out=bqk_sb, in_=bqk[:, :])
        nc.sync.dma_start(out=masks_sb, in_=masks3)
        bv_b = bass.AP(tensor=bv[:, :].tensor, offset=bv[:, :].offset,
                       ap=[[0, 128], [1, H]])
        nc.sync.dma_start(out=bv_bc, in_=bv_b)
        nc.sync.dma_start(out=wall_sb, in_=wall3)

        def x_dma(lo, n):
            nc.sync.dma_start(
                out=xt_all[:, :, lo : lo + n], in_=xT3[:, :, lo : lo + n]
            )

        # persistent SBUF state; slot (0, j) = my block j, (1, j) = peer's
        qT_sb = singles.tile([H, NMYB, 128], BF16)
        kT_sb = singles.tile([H, 2, NMYB, 128], BF16)
        v_sb = singles.tile([128, 2, NMYB, 65], BF16)
        out_stage = singles.tile([128, NMYB, H], F32)
        rcp_sb = singles.tile([128, NMYB], F32)
        nc.vector.memset(v_sb[:, :, :, H : H + 1], 1.0)

        pj = ctx.enter_context(tc.tile_pool(name="pj", bufs=2, space="PSUM"))
        ps = ctx.enter_context(tc.tile_pool(name="ps", bufs=2, space="PSUM"))
        pacc = ctx.enter_context(tc.tile_pool(name="pacc", bufs=2, space="PSUM"))
        pexp_pool = ctx.enter_context(tc.tile_pool(name="pexp", bufs=5))
        vpt = ctx.enter_context(tc.tile_pool(name="vpt", bufs=2))

        def proj_mine(p):
            """q/k/v for my token blocks 2p, 2p+1 (xt cols 256p..)."""
            lo = 256 * p
            b0 = 2 * p
            # [q|k] 0:256 | v_nat b0 256:320 | v_nat b0+1 320:384
            pjt = pj.tile([128, 384], F32, tag="pj", name="pjt")
            for ch in range(NCH):
                nc.tensor.matmul(
                    pjt[:, 0:256], lhsT=wall_sb[:, ch, 0 : 2 * H],
                    rhs=xt_all[:, ch, lo : lo + 256],
                    start=(ch == 0), stop=(ch == NCH - 1),
                )
            for b2 in range(2):
                for ch in range(NCH):
                    nc.tensor.matmul(
                        pjt[:, 256 + 64 * b2 : 320 + 64 * b2],
                        lhsT=xt_all[:, ch, lo + 128 * b2 : lo + 128 * b2 + 128],
                        rhs=wall_sb[:, ch, 2 * H : 3 * H],
                        start=(ch == 0), stop=(ch == NCH - 1),
                    )
            nc.vector.tensor_scalar_add(
                qT_sb[:, b0 : b0 + 2, :], pjt[0:H, 0:256], bqk_sb[0:H, :]
            )
            nc.vector.tensor_scalar_add(
                kT_sb[:, 0, b0 : b0 + 2, :], pjt[H : 2 * H, 0:256],
                bqk_sb[H : 2 * H, :],
            )
            nc.vector.tensor_add(v_sb[:, 0, b0, 0:H], pjt[:, 256:320], bv_bc)
            nc.vector.tensor_add(v_sb[:, 0, b0 + 1, 0:H], pjt[:, 320:384], bv_bc)

        def proj_peer(p):
            """k/vT for peer token blocks 2p, 2p+1 (xt cols 2048+256p..)."""
            lo = S // 2 + 256 * p
            b0 = 2 * p
            # [k|vT] 0:256 | v_nat b0 256:320 | v_nat b0+1 320:384
            pjt = pj.tile([128, 384], F32, tag="pj", name="pjt")
            for ch in range(NCH):
                nc.tensor.matmul(
                    pjt[:, 0:256], lhsT=wall_sb[:, ch, H : 3 * H],
                    rhs=xt_all[:, ch, lo : lo + 256],
                    start=(ch == 0), stop=(ch == NCH - 1),
                )
            vpt_t = vpt.tile([H, 2, 128], F32, tag="vpt", name="vpt_t")
            nc.vector.tensor_scalar_add(
                kT_sb[:, 1, b0 : b0 + 2, :], pjt[0:H, 0:256],
                bqk_sb[H : 2 * H, :],
            )
            nc.vector.tensor_copy(vpt_t, pjt[H : 2 * H, 0:256])
            for b2 in range(2):
                nc.tensor.transpose(
                    pjt[:, 256 + 64 * b2 : 320 + 64 * b2], vpt_t[:, b2, :],
                    ident64,
                )
                nc.vector.tensor_add(
                    v_sb[:, 1, b0 + b2, 0:H],
                    pjt[:, 256 + 64 * b2 : 320 + 64 * b2], bv_bc,
                )

        # ---- attention pipeline ----
        cur = {"ps": None, "n": 0, "meta": []}
        deferred = []  # (ps_tile, pexp_tile, meta) awaiting out-matmul emission
        acc_of = {}

        def get_acc(i):
            if i not in acc_of:
                t = pacc.tile([128, 2, 66], F32, tag="pacc", name="acc_t")
                acc_of[i] = t
                acc_of[i + 1] = t
            return acc_of[i]

        def emit_out(batch):
            """Out matmuls (and norms) for a completed exp tile."""
            ps_t, px_t, meta = batch
            for g, (i, r, j) in enumerate(meta):
                nc.tensor.matmul(
                    acc_of[i][:, i % 2, 0:65],
                    lhsT=px_t[:, g, :],
                    rhs=v_sb[:, r, j, :],
                    start=(r == 0 and j == 0),
                    stop=(r == 1 and j == i),
                )
                if r == 1 and j == i:
                    nc.vector.reciprocal(
                        rcp_sb[:, i : i + 1], acc_of[i][:, i % 2, 64:65]
                    )
                    nc.vector.tensor_scalar_mul(
                        out_stage[:, i, :], acc_of[i][:, i % 2, 0:64],
                        rcp_sb[:, i : i + 1],
                    )
                    if i % 4 == 3:
                        g4 = i // 4
                        nc.gpsimd.dma_start(
                            out=out3[:, 4 * g4 : 4 * g4 + 4, :],
                            in_=out_stage[:, 4 * g4 : 4 * g4 + 4, :],
                        )

        def flush_tile():
            """Close the current scores tile: exp it, queue its out matmuls."""
            if cur["n"] == 0:
                return
            ps_t, meta = cur["ps"], cur["meta"]
            px_t = pexp_pool.tile([128, GPT, 128], BF16, tag="px", name="px_t")
            n = cur["n"]
            nc.scalar.activation(
                px_t[:, 0:n, :], ps_t[:, 0:n, :],
                mybir.ActivationFunctionType.Exp, scale=SCALE,
            )
            deferred.append((ps_t, px_t, list(meta)))
            cur["ps"], cur["n"], cur["meta"] = None, 0, []
            if len(deferred) > 3:
                emit_out(deferred.pop(0))

        FILL = 0

        def group(i, r, j):
            """Scores (+mask) for q block i vs k slot (r, j)."""
            if cur["ps"] is None:
                cur["ps"] = ps.tile([128, GPT, 128], F32, tag="ps", name="ps_t")
                # filler matmuls: keep the tensor engine p-state ramped while
                # it waits for exp to free the ring; overwritten by start=True
                for f in range(FILL):
                    nc.tensor.matmul(
                        cur["ps"][:, 2 * f : 2 * f + 2, :],
                        lhsT=ident_sb, rhs=masks_sb[:, :, :],
                        start=True, stop=True,
                    )
            g = cur["n"]
            ps_t = cur["ps"]
            masked = j == i
            nc.tensor.matmul(
                ps_t[:, g, :], lhsT=kT_sb[:, r, j, :], rhs=qT_sb[:, i, :],
                start=True, stop=not masked,
            )
            if masked:
                nc.tensor.matmul(
                    ps_t[:, g, :], lhsT=ident_sb, rhs=masks_sb[:, r, :],
                    start=False, stop=True,
                )
            cur["meta"].append((i, r, j))
            cur["n"] += 1
            if cur["n"] == GPT:
                flush_tile()

        def wave(w):
            """Attention for my q blocks 2w, 2w+1 (k slots j <= i ready)."""
            get_acc(2 * w)
            for i in (2 * w, 2 * w + 1):
                for j in range(i + 1):
                    for r in range(2):
                        group(i, r, j)

        # ---- pipelined emission ----
        x_dma(0, 256)           # my blocks 0-1
        x_dma(S // 2, 256)      # peer blocks 0-1

        # PE warmup: dependency-free matmuls so the tensor engine is at full
        # p-state when proj 0's data lands
        wps = ps.tile([128, GPT, 128], F32, tag="ps", name="wps")
        for wi in range(24):
            nc.tensor.matmul(
                wps[:, wi % GPT, :], lhsT=ident_sb, rhs=ident_sb,
                start=True, stop=True,
            )

        for p in range(8):
            proj_mine(p)
            proj_peer(p)
            if p < 7:
                x_dma(256 * (p + 1), 256)
                x_dma(S // 2 + 256 * (p + 1), 256)
            if p >= 1:
                wave(p - 1)
                if p == 1:
                    flush_tile()  # release wave 0 without waiting on wave 1
        wave(6)
        wave(7)
        flush_tile()
        while deferred:
            emit_out(deferred.pop(0))

    nc.finalize()
    return nc


_PROGRAM_CACHE = {}


def _get_program():
    if "prog" not in _PROGRAM_CACHE:
        _PROGRAM_CACHE["prog"] = build_program()
    return _PROGRAM_CACHE["prog"]


def _bf16(a):
    import ml_dtypes
    return np.asarray(a).astype(ml_dtypes.bfloat16)


def _make_masks(parity: int) -> np.ndarray:
    """[2, 128, 128] additive mask tiles for the j == i k slot.

    Slot (0, i) is my own block i = the diagonal (strict lower triangle of
    scores^T masked: k row p > q col j).  Slot (1, i) is the peer's block i:
    natural 2i+1 > 2i for even cores (fully masked), natural 2i < 2i+1 for
    odd cores (fully valid).
    """
    p = np.arange(128)[:, None]
    j = np.arange(128)[None, :]
    tri = np.where(p > j, MASK_VAL, 0.0).astype(np.float32)
    full = np.full((128, 128), MASK_VAL, np.float32)
    zero = np.zeros((128, 128), np.float32)
    m = np.stack([tri, full if parity == 0 else zero])
    return _bf16(m)


def kernel(x, Wq, bq, Wk, bk, Wv, bv):
    x = np.asarray(x, dtype=np.float32)
    wall = np.zeros((D, 256), np.float32)
    wall[:, 0:H] = np.asarray(Wq)
    wall[:, H : 2 * H] = np.asarray(Wk)
    wall[:, 2 * H : 3 * H] = np.asarray(Wv)
    wall = _bf16(wall)
    bqk = np.concatenate(
        [np.asarray(bq), np.asarray(bk)]
    ).astype(np.float32).reshape(2 * H, 1)
    bv_ = np.asarray(bv, dtype=np.float32).reshape(1, H)
    ident = _bf16(np.eye(128, dtype=np.float32))

    nc = _get_program()

    in_maps = []
    for core in range(NCORES):
        b, parity = core // 2, core % 2
        xb = x[b].T.reshape(D, 32, 128)
        mine = xb[:, parity::2, :].reshape(D, S // 2)
        peer = xb[:, 1 - parity :: 2, :].reshape(D, S // 2)
        xTp = np.ascontiguousarray(np.concatenate([mine, peer], axis=1))
        in_maps.append(
            {
                "xT": _bf16(xTp),
                "wall": wall,
                "bqk": bqk,
                "bv": bv_,
                "masks": _make_masks(parity),
                "ident": ident,
            }
        )

    res = run_bass_kernel_spmd(nc, in_maps, list(range(NCORES)))

    out = np.empty((B, S, H), np.float32)
    for core in range(NCORES):
        b, parity = core // 2, core % 2
        o = np.asarray(res.results[core]["out"], np.float32).reshape(NMYB, 128, H)
        out[b].reshape(32, 128, H)[parity::2] = o
    return out


# revision 21
# speedup vs baseline: 1.2286x; 1.0156x over previous
"""Causal single-head attention (B=4, S=4096, D=1024, H=64) on 8 TRN2 NeuronCores.

Strategy (v3: sequence-parallel, peer k/v computed locally)
-----------------------------------------------------------
Data-parallel over batch (2 cores per batch element); within a pair the q rows
are split by 128-row block parity (even core: natural blocks 0,2,..,30; odd:
1,3,..,31), which load-balances the causal triangle.  Each core loads BOTH
halves of its batch element's activations (bf16, 8 MB) and computes q for its
own blocks plus k/v for ALL 32 blocks locally — no collectives, no exchange:
every dependency in the attention pipeline is a short on-core PE->DVE->PE hop.
Each core then produces COMPLETE attention rows for its own q blocks, so the
softmax denominator is a per-partition scalar and outputs are written directly.

Layouts chosen for the TRN2 cost model (matmul cost = moving columns only):
- All matmuls bf16 (1 col/cycle at any width; f32 weights would cost 4x).
- Projections: my tokens stream through lhsT=[Wq|Wk] (128 wide); peer tokens
  stream through lhsT=[Wk|Wv] — a contiguous slice of the same packed weight
  wall.  My v is projected directly in natural [token, h] layout (x chunk as
  lhsT, Wv moving: 64 cols per block); peer v arrives transposed in the
  [Wk|Wv] pass and is turned natural by a PE transpose (64 cols each).
- kT is [h, kpos] so scores^T tiles [128 k, 128 q] come out k-on-partitions,
  which is exactly the lhsT layout the out matmul wants:
  out_acc[128 q, 65] += lhsT=exp_tile, rhs=v_nat[128 k, 65] — only 65 moving
  cols per (k block, q block).  Col 64 of v_nat is constant 1, so the softmax
  denominator accumulates as output column 64 for free.
- The causal mask is applied ON the tensor engine: an extra accumulating
  matmul lhsT=I, rhs=mask_tile adds -8e9 into the banded scores PSUM.
  Host-computed mask data absorbs the even/odd parity asymmetry (the SPMD
  instruction stream is identical on all 8 cores): slot (0, j) holds my
  parity's block j, slot (1, j) the peer's; for q block i, slot (0, i) is
  always the diagonal (triangular mask) and slot (1, i) is fully masked on
  even cores / fully valid on odd cores, so extents are uniformly 2i+2.
- exp runs on the scalar engine out of PSUM in [128, 8*128] tiles (35 ops
  total) with the 1/8 scale folded in; masked entries underflow to exactly 0,
  matching the reference's -1e9 semantics.  exp outputs bf16; the scalar
  queue carries ONLY exp so it never sits behind a waiting DMA.
- Normalization: out_acc col 64 is the denominator (per-partition scalar) ->
  DVE reciprocal + tensor_scalar_mul, then direct DMA of the final rows.

The host only does layout work plus the fp32->bf16 cast of the inputs
(~5e-3 worst-case relative error, well inside the 2e-2 gate).
"""

import numpy as np
from contextlib import ExitStack

import concourse.bass as bass
import concourse.mybir as mybir
import concourse.tile as tile
from concourse import bacc
from concourse.bass_utils import run_bass_kernel_spmd
from concourse.masks import make_identity

F32 = mybir.dt.float32
BF16 = mybir.dt.bfloat16

B, S, D, H = 4, 4096, 1024, 64
NCORES = 8
NCH = D // 128        # contraction chunks
NMYB = 16             # my q blocks per core
MASK_VAL = -8.0e9     # raw-score mask; exp(0.125*(s+MASK_VAL)) == 0
SCALE = 0.125         # 1/sqrt(H)
GPT = 8               # score/exp groups per PSUM tile ([128, GPT*128])


def build_program(with_cc: bool = True):
    nc = bacc.Bacc(num_devices=NCORES)

    xT = nc.declare_dram_parameter("xT", [D, S], BF16, isOutput=False)
    wall = nc.declare_dram_parameter("wall", [D, 256], BF16, isOutput=False)
    bqk = nc.declare_dram_parameter("bqk", [2 * H, 1], F32, isOutput=False)
    bv = nc.declare_dram_parameter("bv", [1, H], F32, isOutput=False)
    masks = nc.declare_dram_parameter("masks", [2, 128, 128], BF16, isOutput=False)
    ident = nc.declare_dram_parameter("ident", [128, 128], BF16, isOutput=False)
    out = nc.declare_dram_parameter("out", [S // 2, H], F32, isOutput=True)

    xT3 = xT.rearrange("(c p) s -> p c s", p=128)        # [128, 8, 4096]
    wall3 = wall.rearrange("(c p) h -> p c h", p=128)    # [128, 8, 256]
    masks3 = masks.rearrange("m p j -> p m j")           # [128, 2, 128]
    out3 = out.rearrange("(n p) h -> p n h", p=128)      # [128, 16, 64]

    with ExitStack() as ctx:
        tc = ctx.enter_context(tile.TileContext(nc))

        singles = ctx.enter_context(tc.tile_pool(name="singles", bufs=1))

        xt_all = singles.tile([128, NCH, S], BF16)
        wall_sb = singles.tile([128, NCH, 256], BF16)
        bqk_sb = singles.tile([2 * H, 1], F32)
        bv_bc = singles.tile([128, H], F32)
        masks_sb = singles.tile([128, 2, 128], BF16)
        ident_sb = singles.tile([128, 128], BF16)
        ident64 = singles.tile([H, H], F32)
        make_identity(nc, ident64)

        # priority order on the sync queue: tiny params, weights, then the x
        # stream (my/peer piece pairs in consumption order)
        nc.sync.dma_start(out=ident_sb, in_=ident[:, :])
        nc.sync.dma_start(out=bqk_sb, in_=bqk[:, :])
        nc.sync.dma_start(out=masks_sb, in_=masks3)
        bv_b = bass.AP(tensor=bv[:, :].tensor, offset=bv[:, :].offset,
                       ap=[[0, 128], [1, H]])
        nc.sync.dma_start(out=bv_bc, in_=bv_b)
        nc.sync.dma_start(out=wall_sb, in_=wall3)

        def x_dma(lo, n):
            nc.sync.dma_start(
                out=xt_all[:, :, lo : lo + n], in_=xT3[:, :, lo : lo + n]
            )

        # persistent SBUF state; slot (0, j) = my block j, (1, j) = peer's
        qT_sb = singles.tile([H, NMYB, 128], BF16)
        kT_sb = singles.tile([H, 2, NMYB, 128], BF16)
        v_sb = singles.tile([128, 2, NMYB, 65], BF16)
        out_stage = singles.tile([128, NMYB, H], F32)
        rcp_sb = singles.tile([128, NMYB], F32)
        nc.vector.memset(v_sb[:, :, :, H : H + 1], 1.0)

        pj = ctx.enter_context(tc.tile_pool(name="pj", bufs=2, space="PSUM"))
        ps = ctx.enter_context(tc.tile_pool(name="ps", bufs=2, space="PSUM"))
        pacc = ctx.enter_context(tc.tile_pool(name="pacc", bufs=2, space="PSUM"))
        pexp_pool = ctx.enter_context(tc.tile_pool(name="pexp", bufs=5))
        vpt = ctx.enter_context(tc.tile_pool(name="vpt", bufs=2))

        def proj_mine(p):
            """q/k/v for my token blocks 2p, 2p+1 (xt cols 256p..)."""
            lo = 256 * p
            b0 = 2 * p
            # [q|k] 0:256 | v_nat b0 256:320 | v_nat b0+1 320:384
            pjt = pj.tile([128, 384], F32, tag="pj", name="pjt")
            for ch in range(NCH):
                nc.tensor.matmul(
                    pjt[:, 0:256], lhsT=wall_sb[:, ch, 0 : 2 * H],
                    rhs=xt_all[:, ch, lo : lo + 256],
                    start=(ch == 0), stop=(ch == NCH - 1),
                )
            for b2 in range(2):
                for ch in range(NCH):
                    nc.tensor.matmul(
                        pjt[:, 256 + 64 * b2 : 320 + 64 * b2],
                        lhsT=xt_all[:, ch, lo + 128 * b2 : lo + 128 * b2 + 128],
                        rhs=wall_sb[:, ch, 2 * H : 3 * H],
                        start=(ch == 0), stop=(ch == NCH - 1),
                    )
            nc.vector.tensor_scalar_add(
                qT_sb[:, b0 : b0 + 2, :], pjt[0:H, 0:256], bqk_sb[0:H, :]
            )
            nc.vector.tensor_scalar_add(
                kT_sb[:, 0, b0 : b0 + 2, :], pjt[H : 2 * H, 0:256],
                bqk_sb[H : 2 * H, :],
            )
            nc.vector.tensor_add(v_sb[:, 0, b0, 0:H], pjt[:, 256:320], bv_bc)
            nc.vector.tensor_add(v_sb[:, 0, b0 + 1, 0:H], pjt[:, 320:384], bv_bc)

        def proj_peer(p):
            """k/vT for peer token blocks 2p, 2p+1 (xt cols 2048+256p..)."""
            lo = S // 2 + 256 * p
            b0 = 2 * p
            # [k|vT] 0:256 | v_nat b0 256:320 | v_nat b0+1 320:384
            pjt = pj.tile([128, 384], F32, tag="pj", name="pjt")
            for ch in range(NCH):
                nc.tensor.matmul(
                    pjt[:, 0:256], lhsT=wall_sb[:, ch, H : 3 * H],
                    rhs=xt_all[:, ch, lo : lo + 256],
                    start=(ch == 0), stop=(ch == NCH - 1),
                )
            vpt_t = vpt.tile([H, 2, 128], F32, tag="vpt", name="vpt_t")
            nc.vector.tensor_scalar_add(
                kT_sb[:, 1, b0 : b0 + 2, :], pjt[0:H, 0:256],
                bqk_sb[H : 2 * H, :],
            )
            nc.vector.tensor_copy(vpt_t, pjt[H : 2 * H, 0:256])
            for b2 in range(2):
                nc.tensor.transpose(
                    pjt[:, 256 + 64 * b2 : 320 + 64 * b2], vpt_t[:, b2, :],
                    ident64,
                )
                nc.vector.tensor_add(
                    v_sb[:, 1, b0 + b2, 0:H],
                    pjt[:, 256 + 64 * b2 : 320 + 64 * b2], bv_bc,
                )

        # ---- attention pipeline ----
        cur = {"ps": None, "n": 0, "meta": []}
        deferred = []  # (ps_tile, pexp_tile, meta) awaiting out-matmul emission
        acc_of = {}

        def get_acc(i):
            if i not in acc_of:
                t = pacc.tile([128, 2, 66], F32, tag="pacc", name="acc_t")
                acc_of[i] = t
                acc_of[i + 1] = t
            return acc_of[i]

        def emit_out(batch):
            """Out matmuls (and norms) for a completed exp tile."""
            ps_t, px_t, meta = batch
            for g, (i, r, j) in enumerate(meta):
                nc.tensor.matmul(
                    acc_of[i][:, i % 2, 0:65],
                    lhsT=px_t[:, g, :],
                    rhs=v_sb[:, r, j, :],
                    start=(r == 0 and j == 0),
                    stop=(r == 1 and j == i),
                )
                if r == 1 and j == i:
                    nc.vector.reciprocal(
                        rcp_sb[:, i : i + 1], acc_of[i][:, i % 2, 64:65]
                    )
                    nc.vector.tensor_scalar_mul(
                        out_stage[:, i, :], acc_of[i][:, i % 2, 0:64],
                        rcp_sb[:, i : i + 1],
                    )
                    # final rows: progressively finer pieces on the sync
                    # queue so the last norm->write->drain tail is short
                    if i == 3 or i == 7 or i == 11:
                        g4 = i // 4
                        nc.gpsimd.dma_start(
                            out=out3[:, 4 * g4 : 4 * g4 + 4, :],
                            in_=out_stage[:, 4 * g4 : 4 * g4 + 4, :],
                        )
                    elif i == 13:
                        nc.sync.dma_start(
                            out=out3[:, 12:14, :], in_=out_stage[:, 12:14, :]
                        )
                    elif i >= 14:
                        nc.sync.dma_start(
                            out=out3[:, i : i + 1, :],
                            in_=out_stage[:, i : i + 1, :],
                        )

        def flush_tile():
            """Close the current scores tile: exp it, queue its out matmuls."""
            if cur["n"] == 0:
                return
            ps_t, meta = cur["ps"], cur["meta"]
            px_t = pexp_pool.tile([128, GPT, 128], BF16, tag="px", name="px_t")
            n = cur["n"]
            nc.scalar.activation(
                px_t[:, 0:n, :], ps_t[:, 0:n, :],
                mybir.ActivationFunctionType.Exp, scale=SCALE,
            )
            deferred.append((ps_t, px_t, list(meta)))
            cur["ps"], cur["n"], cur["meta"] = None, 0, []
            if len(deferred) > 3:
                emit_out(deferred.pop(0))

        FILL = 0

        def group(i, r, j):
            """Scores (+mask) for q block i vs k slot (r, j)."""
            if cur["ps"] is None:
                cur["ps"] = ps.tile([128, GPT, 128], F32, tag="ps", name="ps_t")
                # filler matmuls: keep the tensor engine p-state ramped while
                # it waits for exp to free the ring; overwritten by start=True
                for f in range(FILL):
                    nc.tensor.matmul(
                        cur["ps"][:, 2 * f : 2 * f + 2, :],
                        lhsT=ident_sb, rhs=masks_sb[:, :, :],
                        start=True, stop=True,
                    )
            g = cur["n"]
            ps_t = cur["ps"]
            masked = j == i
            nc.tensor.matmul(
                ps_t[:, g, :], lhsT=kT_sb[:, r, j, :], rhs=qT_sb[:, i, :],
                start=True, stop=not masked,
            )
            if masked:
                nc.tensor.matmul(
                    ps_t[:, g, :], lhsT=ident_sb, rhs=masks_sb[:, r, :],
                    start=False, stop=True,
                )
            cur["meta"].append((i, r, j))
            cur["n"] += 1
            if cur["n"] == GPT:
                flush_tile()

        def wave(w):
            """Attention for my q blocks 2w, 2w+1 (k slots j <= i ready)."""
            get_acc(2 * w)
            for i in (2 * w, 2 * w + 1):
                for j in range(i + 1):
                    for r in range(2):
                        group(i, r, j)

        # ---- pipelined emission ----
        x_dma(0, 256)           # my blocks 0-1
        x_dma(S // 2, 256)      # peer blocks 0-1

        # PE warmup: dependency-free matmuls so the tensor engine is at full
        # p-state when proj 0's data lands
        wps = ps.tile([128, GPT, 128], F32, tag="ps", name="wps")
        for wi in range(24):
            nc.tensor.matmul(
                wps[:, wi % GPT, :], lhsT=ident_sb, rhs=ident_sb,
                start=True, stop=True,
            )

        for p in range(8):
            proj_mine(p)
            proj_peer(p)
            if p < 7:
                x_dma(256 * (p + 1), 256)
                x_dma(S // 2 + 256 * (p + 1), 256)
            if p >= 1:
                wave(p - 1)
                if p == 1:
                    flush_tile()  # release wave 0 without waiting on wave 1
        wave(6)
        wave(7)
        flush_tile()
        while deferred:
            emit_out(deferred.pop(0))

    nc.finalize()
    return nc


_PROGRAM_CACHE = {}


def _get_program():
    if "prog" not in _PROGRAM_CACHE:
        _PROGRAM_CACHE["prog"] = build_program()
    return _PROGRAM_CACHE["prog"]


def _bf16(a):
    import ml_dtypes
    return np.asarray(a).astype(ml_dtypes.bfloat16)


def _make_masks(parity: int) -> np.ndarray:
    """[2, 128, 128] additive mask tiles for the j == i k slot.

    Slot (0, i) is my own block i = the diagonal (strict lower triangle of
    scores^T masked: k row p > q col j).  Slot (1, i) is the peer's block i:
    natural 2i+1 > 2i for even cores (fully masked), natural 2i < 2i+1 for
    odd cores (fully valid).
    """
    p = np.arange(128)[:, None]
    j = np.arange(128)[None, :]
    tri = np.where(p > j, MASK_VAL, 0.0).astype(np.float32)
    full = np.full((128, 128), MASK_VAL, np.float32)
    zero = np.zeros((128, 128), np.float32)
    m = np.stack([tri, full if parity == 0 else zero])
    return _bf16(m)


def kernel(x, Wq, bq, Wk, bk, Wv, bv):
    x = np.asarray(x, dtype=np.float32)
    wall = np.zeros((D, 256), np.float32)
    wall[:, 0:H] = np.asarray(Wq)
    wall[:, H : 2 * H] = np.asarray(Wk)
    wall[:, 2 * H : 3 * H] = np.asarray(Wv)
    wall = _bf16(wall)
    bqk = np.concatenate(
        [np.asarray(bq), np.asarray(bk)]
    ).astype(np.float32).reshape(2 * H, 1)
    bv_ = np.asarray(bv, dtype=np.float32).reshape(1, H)
    ident = _bf16(np.eye(128, dtype=np.float32))

    nc = _get_program()

    in_maps = []
    for core in range(NCORES):
        b, parity = core // 2, core % 2
        xb = x[b].T.reshape(D, 32, 128)
        mine = xb[:, parity::2, :].reshape(D, S // 2)
        peer = xb[:, 1 - parity :: 2, :].reshape(D, S // 2)
        xTp = np.ascontiguousarray(np.concatenate([mine, peer], axis=1))
        in_maps.append(
            {
                "xT": _bf16(xTp),
                "wall": wall,
                "bqk": bqk,
                "bv": bv_,
                "masks": _make_masks(parity),
                "ident": ident,
            }
        )

    res = run_bass_kernel_spmd(nc, in_maps, list(range(NCORES)))

    out = np.empty((B, S, H), np.float32)
    for core in range(NCORES):
        b, parity = core // 2, core % 2
        o = np.asarray(res.results[core]["out"], np.float32).reshape(NMYB, 128, H)
        out[b].reshape(32, 128, H)[parity::2] = o
    return out


# revision 22
# speedup vs baseline: 1.2303x; 1.0014x over previous
"""Causal single-head attention (B=4, S=4096, D=1024, H=64) on 8 TRN2 NeuronCores.

Strategy (v3: sequence-parallel, peer k/v computed locally)
-----------------------------------------------------------
Data-parallel over batch (2 cores per batch element); within a pair the q rows
are split by 128-row block parity (even core: natural blocks 0,2,..,30; odd:
1,3,..,31), which load-balances the causal triangle.  Each core loads BOTH
halves of its batch element's activations (bf16, 8 MB) and computes q for its
own blocks plus k/v for ALL 32 blocks locally — no collectives, no exchange:
every dependency in the attention pipeline is a short on-core PE->DVE->PE hop.
Each core then produces COMPLETE attention rows for its own q blocks, so the
softmax denominator is a per-partition scalar and outputs are written directly.

Layouts chosen for the TRN2 cost model (matmul cost = moving columns only):
- All matmuls bf16 (1 col/cycle at any width; f32 weights would cost 4x).
- Projections: my tokens stream through lhsT=[Wq|Wk] (128 wide); peer tokens
  stream through lhsT=[Wk|Wv] — a contiguous slice of the same packed weight
  wall.  My v is projected directly in natural [token, h] layout (x chunk as
  lhsT, Wv moving: 64 cols per block); peer v arrives transposed in the
  [Wk|Wv] pass and is turned natural by a PE transpose (64 cols each).
- kT is [h, kpos] so scores^T tiles [128 k, 128 q] come out k-on-partitions,
  which is exactly the lhsT layout the out matmul wants:
  out_acc[128 q, 65] += lhsT=exp_tile, rhs=v_nat[128 k, 65] — only 65 moving
  cols per (k block, q block).  Col 64 of v_nat is constant 1, so the softmax
  denominator accumulates as output column 64 for free.
- The causal mask is applied ON the tensor engine: an extra accumulating
  matmul lhsT=I, rhs=mask_tile adds -8e9 into the banded scores PSUM.
  Host-computed mask data absorbs the even/odd parity asymmetry (the SPMD
  instruction stream is identical on all 8 cores): slot (0, j) holds my
  parity's block j, slot (1, j) the peer's; for q block i, slot (0, i) is
  always the diagonal (triangular mask) and slot (1, i) is fully masked on
  even cores / fully valid on odd cores, so extents are uniformly 2i+2.
- exp runs on the scalar engine out of PSUM in [128, 8*128] tiles (35 ops
  total) with the 1/8 scale folded in; masked entries underflow to exactly 0,
  matching the reference's -1e9 semantics.  exp outputs bf16; the scalar
  queue carries ONLY exp so it never sits behind a waiting DMA.
- Normalization: out_acc col 64 is the denominator (per-partition scalar) ->
  DVE reciprocal + tensor_scalar_mul, then direct DMA of the final rows.

The host only does layout work plus the fp32->bf16 cast of the inputs
(~5e-3 worst-case relative error, well inside the 2e-2 gate).
"""

import numpy as np
from contextlib import ExitStack

import concourse.bass as bass
import concourse.mybir as mybir
import concourse.tile as tile
from concourse import bacc
from concourse.bass_utils import run_bass_kernel_spmd
from concourse.masks import make_identity

F32 = mybir.dt.float32
BF16 = mybir.dt.bfloat16

B, S, D, H = 4, 4096, 1024, 64
NCORES = 8
NCH = D // 128        # contraction chunks
NMYB = 16             # my q blocks per core
MASK_VAL = -8.0e9     # raw-score mask; exp(0.125*(s+MASK_VAL)) == 0
SCALE = 0.125         # 1/sqrt(H)
GPT = 8               # score/exp groups per PSUM tile ([128, GPT*128])


def build_program(with_cc: bool = True):
    nc = bacc.Bacc(num_devices=NCORES)

    xT = nc.declare_dram_parameter("xT", [D, S], BF16, isOutput=False)
    wall = nc.declare_dram_parameter("wall", [D, 256], BF16, isOutput=False)
    bqk = nc.declare_dram_parameter("bqk", [2 * H, 1], F32, isOutput=False)
    bv = nc.declare_dram_parameter("bv", [1, H], F32, isOutput=False)
    masks = nc.declare_dram_parameter("masks", [2, 128, 128], BF16, isOutput=False)
    ident = nc.declare_dram_parameter("ident", [128, 128], BF16, isOutput=False)
    out = nc.declare_dram_parameter("out", [S // 2, H], F32, isOutput=True)

    xT3 = xT.rearrange("(c p) s -> p c s", p=128)        # [128, 8, 4096]
    wall3 = wall.rearrange("(c p) h -> p c h", p=128)    # [128, 8, 256]
    masks3 = masks.rearrange("m p j -> p m j")           # [128, 2, 128]
    out3 = out.rearrange("(n p) h -> p n h", p=128)      # [128, 16, 64]

    with ExitStack() as ctx:
        tc = ctx.enter_context(tile.TileContext(nc))

        singles = ctx.enter_context(tc.tile_pool(name="singles", bufs=1))

        xt_all = singles.tile([128, NCH, S], BF16)
        wall_sb = singles.tile([128, NCH, 256], BF16)
        bqk_sb = singles.tile([2 * H, 1], F32)
        bv_bc = singles.tile([128, H], F32)
        masks_sb = singles.tile([128, 2, 128], BF16)
        ident_sb = singles.tile([128, 128], BF16)
        ident64 = singles.tile([H, H], F32)
        make_identity(nc, ident64)

        # priority order on the sync queue: tiny params, weights, then the x
        # stream (my/peer piece pairs in consumption order)
        nc.sync.dma_start(out=ident_sb, in_=ident[:, :])
        nc.sync.dma_start(out=bqk_sb, in_=bqk[:, :])
        nc.sync.dma_start(out=masks_sb, in_=masks3)
        bv_b = bass.AP(tensor=bv[:, :].tensor, offset=bv[:, :].offset,
                       ap=[[0, 128], [1, H]])
        nc.sync.dma_start(out=bv_bc, in_=bv_b)
        nc.sync.dma_start(out=wall_sb, in_=wall3)

        def x_dma(lo, n):
            nc.sync.dma_start(
                out=xt_all[:, :, lo : lo + n], in_=xT3[:, :, lo : lo + n]
            )

        # persistent SBUF state; slot (0, j) = my block j, (1, j) = peer's
        qT_sb = singles.tile([H, NMYB, 128], BF16)
        kT_sb = singles.tile([H, 2, NMYB, 128], BF16)
        v_sb = singles.tile([128, 2, NMYB, 65], BF16)
        out_stage = singles.tile([128, NMYB, H], F32)
        rcp_sb = singles.tile([128, NMYB], F32)
        nc.vector.memset(v_sb[:, :, :, H : H + 1], 1.0)

        pj = ctx.enter_context(tc.tile_pool(name="pj", bufs=2, space="PSUM"))
        ps = ctx.enter_context(tc.tile_pool(name="ps", bufs=2, space="PSUM"))
        pacc = ctx.enter_context(tc.tile_pool(name="pacc", bufs=2, space="PSUM"))
        pexp_pool = ctx.enter_context(tc.tile_pool(name="pexp", bufs=5))
        vpt = ctx.enter_context(tc.tile_pool(name="vpt", bufs=2))

        def proj_mine(p):
            """q/k/v for my token blocks 2p, 2p+1 (xt cols 256p..)."""
            lo = 256 * p
            b0 = 2 * p
            # [q|k] 0:256 | v_nat b0 256:320 | v_nat b0+1 320:384
            pjt = pj.tile([128, 384], F32, tag="pj", name="pjt")
            for ch in range(NCH):
                nc.tensor.matmul(
                    pjt[:, 0:256], lhsT=wall_sb[:, ch, 0 : 2 * H],
                    rhs=xt_all[:, ch, lo : lo + 256],
                    start=(ch == 0), stop=(ch == NCH - 1),
                )
            for b2 in range(2):
                for ch in range(NCH):
                    nc.tensor.matmul(
                        pjt[:, 256 + 64 * b2 : 320 + 64 * b2],
                        lhsT=xt_all[:, ch, lo + 128 * b2 : lo + 128 * b2 + 128],
                        rhs=wall_sb[:, ch, 2 * H : 3 * H],
                        start=(ch == 0), stop=(ch == NCH - 1),
                    )
            nc.vector.tensor_scalar_add(
                qT_sb[:, b0 : b0 + 2, :], pjt[0:H, 0:256], bqk_sb[0:H, :]
            )
            nc.vector.tensor_scalar_add(
                kT_sb[:, 0, b0 : b0 + 2, :], pjt[H : 2 * H, 0:256],
                bqk_sb[H : 2 * H, :],
            )
            nc.vector.tensor_add(v_sb[:, 0, b0, 0:H], pjt[:, 256:320], bv_bc)
            nc.vector.tensor_add(v_sb[:, 0, b0 + 1, 0:H], pjt[:, 320:384], bv_bc)

        def proj_peer(p):
            """k/vT for peer token blocks 2p, 2p+1 (xt cols 2048+256p..)."""
            lo = S // 2 + 256 * p
            b0 = 2 * p
            # [k|vT] 0:256 | v_nat b0 256:320 | v_nat b0+1 320:384
            pjt = pj.tile([128, 384], F32, tag="pj", name="pjt")
            for ch in range(NCH):
                nc.tensor.matmul(
                    pjt[:, 0:256], lhsT=wall_sb[:, ch, H : 3 * H],
                    rhs=xt_all[:, ch, lo : lo + 256],
                    start=(ch == 0), stop=(ch == NCH - 1),
                )
            vpt_t = vpt.tile([H, 2, 128], F32, tag="vpt", name="vpt_t")
            nc.vector.tensor_scalar_add(
                kT_sb[:, 1, b0 : b0 + 2, :], pjt[0:H, 0:256],
                bqk_sb[H : 2 * H, :],
            )
            nc.vector.tensor_copy(vpt_t, pjt[H : 2 * H, 0:256])
            for b2 in range(2):
                nc.tensor.transpose(
                    pjt[:, 256 + 64 * b2 : 320 + 64 * b2], vpt_t[:, b2, :],
                    ident64,
                )
                nc.vector.tensor_add(
                    v_sb[:, 1, b0 + b2, 0:H],
                    pjt[:, 256 + 64 * b2 : 320 + 64 * b2], bv_bc,
                )

        # ---- attention pipeline ----
        cur = {"ps": None, "n": 0, "meta": []}
        deferred = []  # (ps_tile, pexp_tile, meta) awaiting out-matmul emission
        acc_of = {}

        def get_acc(i):
            if i not in acc_of:
                t = pacc.tile([128, 2, 66], F32, tag="pacc", name="acc_t")
                acc_of[i] = t
                acc_of[i + 1] = t
            return acc_of[i]

        def emit_out(batch):
            """Out matmuls (and norms) for a completed exp tile."""
            ps_t, px_t, meta = batch
            for g, (i, r, j) in enumerate(meta):
                nc.tensor.matmul(
                    acc_of[i][:, i % 2, 0:65],
                    lhsT=px_t[:, g, :],
                    rhs=v_sb[:, r, j, :],
                    start=(r == 0 and j == 0),
                    stop=(r == 1 and j == i),
                )
                if r == 1 and j == i:
                    nc.vector.reciprocal(
                        rcp_sb[:, i : i + 1], acc_of[i][:, i % 2, 64:65]
                    )
                    nc.vector.tensor_scalar_mul(
                        out_stage[:, i, :], acc_of[i][:, i % 2, 0:64],
                        rcp_sb[:, i : i + 1],
                    )
                    # final rows: progressively finer pieces on the sync
                    # queue so the last norm->write->drain tail is short
                    if i == 3 or i == 7 or i == 11:
                        g4 = i // 4
                        nc.sync.dma_start(
                            out=out3[:, 4 * g4 : 4 * g4 + 4, :],
                            in_=out_stage[:, 4 * g4 : 4 * g4 + 4, :],
                        )
                    elif i == 13:
                        nc.sync.dma_start(
                            out=out3[:, 12:14, :], in_=out_stage[:, 12:14, :]
                        )
                    elif i >= 14:
                        nc.sync.dma_start(
                            out=out3[:, i : i + 1, :],
                            in_=out_stage[:, i : i + 1, :],
                        )

        def flush_tile():
            """Close the current scores tile: exp it, queue its out matmuls."""
            if cur["n"] == 0:
                return
            ps_t, meta = cur["ps"], cur["meta"]
            px_t = pexp_pool.tile([128, GPT, 128], BF16, tag="px", name="px_t")
            n = cur["n"]
            nc.scalar.activation(
                px_t[:, 0:n, :], ps_t[:, 0:n, :],
                mybir.ActivationFunctionType.Exp, scale=SCALE,
            )
            deferred.append((ps_t, px_t, list(meta)))
            cur["ps"], cur["n"], cur["meta"] = None, 0, []
            if len(deferred) > 3:
                emit_out(deferred.pop(0))

        FILL = 0

        def group(i, r, j):
            """Scores (+mask) for q block i vs k slot (r, j)."""
            if cur["ps"] is None:
                cur["ps"] = ps.tile([128, GPT, 128], F32, tag="ps", name="ps_t")
                # filler matmuls: keep the tensor engine p-state ramped while
                # it waits for exp to free the ring; overwritten by start=True
                for f in range(FILL):
                    nc.tensor.matmul(
                        cur["ps"][:, 2 * f : 2 * f + 2, :],
                        lhsT=ident_sb, rhs=masks_sb[:, :, :],
                        start=True, stop=True,
                    )
            g = cur["n"]
            ps_t = cur["ps"]
            masked = j == i
            nc.tensor.matmul(
                ps_t[:, g, :], lhsT=kT_sb[:, r, j, :], rhs=qT_sb[:, i, :],
                start=True, stop=not masked,
            )
            if masked:
                nc.tensor.matmul(
                    ps_t[:, g, :], lhsT=ident_sb, rhs=masks_sb[:, r, :],
                    start=False, stop=True,
                )
            cur["meta"].append((i, r, j))
            cur["n"] += 1
            if cur["n"] == GPT:
                flush_tile()

        def wave(w):
            """Attention for my q blocks 2w, 2w+1 (k slots j <= i ready)."""
            get_acc(2 * w)
            for i in (2 * w, 2 * w + 1):
                for j in range(i + 1):
                    for r in range(2):
                        group(i, r, j)

        # ---- pipelined emission ----
        x_dma(0, 256)           # my blocks 0-1
        x_dma(S // 2, 256)      # peer blocks 0-1

        # PE warmup: dependency-free matmuls so the tensor engine is at full
        # p-state when proj 0's data lands
        wps = ps.tile([128, GPT, 128], F32, tag="ps", name="wps")
        for wi in range(32):
            nc.tensor.matmul(
                wps[:, wi % GPT, :], lhsT=ident_sb, rhs=ident_sb,
                start=True, stop=True,
            )

        for p in range(8):
            proj_mine(p)
            proj_peer(p)
            if p < 7:
                x_dma(256 * (p + 1), 256)
                x_dma(S // 2 + 256 * (p + 1), 256)
            if p >= 1:
                wave(p - 1)
                if p == 1:
                    flush_tile()  # release wave 0 without waiting on wave 1
        wave(6)
        wave(7)
        flush_tile()
        while deferred:
            emit_out(deferred.pop(0))

    nc.finalize()
    return nc


_PROGRAM_CACHE = {}


def _get_program():
    if "prog" not in _PROGRAM_CACHE:
        _PROGRAM_CACHE["prog"] = build_program()
    return _PROGRAM_CACHE["prog"]


def _bf16(a):
    import ml_dtypes
    return np.asarray(a).astype(ml_dtypes.bfloat16)


def _make_masks(parity: int) -> np.ndarray:
    """[2, 128, 128] additive mask tiles for the j == i k slot.

    Slot (0, i) is my own block i = the diagonal (strict lower triangle of
    scores^T masked: k row p > q col j).  Slot (1, i) is the peer's block i:
    natural 2i+1 > 2i for even cores (fully masked), natural 2i < 2i+1 for
    odd cores (fully valid).
    """
    p = np.arange(128)[:, None]
    j = np.arange(128)[None, :]
    tri = np.where(p > j, MASK_VAL, 0.0).astype(np.float32)
    full = np.full((128, 128), MASK_VAL, np.float32)
    zero = np.zeros((128, 128), np.float32)
    m = np.stack([tri, full if parity == 0 else zero])
    return _bf16(m)


def kernel(x, Wq, bq, Wk, bk, Wv, bv):
    x = np.asarray(x, dtype=np.float32)
    wall = np.zeros((D, 256), np.float32)
    wall[:, 0:H] = np.asarray(Wq)
    wall[:, H : 2 * H] = np.asarray(Wk)
    wall[:, 2 * H : 3 * H] = np.asarray(Wv)
    wall = _bf16(wall)
    bqk = np.concatenate(
        [np.asarray(bq), np.asarray(bk)]
    ).astype(np.float32).reshape(2 * H, 1)
    bv_ = np.asarray(bv, dtype=np.float32).reshape(1, H)
    ident = _bf16(np.eye(128, dtype=np.float32))

    nc = _get_program()

    in_maps = []
    for core in range(NCORES):
        b, parity = core // 2, core % 2
        xb = x[b].T.reshape(D, 32, 128)
        mine = xb[:, parity::2, :].reshape(D, S // 2)
        peer = xb[:, 1 - parity :: 2, :].reshape(D, S // 2)
        xTp = np.ascontiguousarray(np.concatenate([mine, peer], axis=1))
        in_maps.append(
            {
                "xT": _bf16(xTp),
                "wall": wall,
                "bqk": bqk,
                "bv": bv_,
                "masks": _make_masks(parity),
                "ident": ident,
            }
        )

    res = run_bass_kernel_spmd(nc, in_maps, list(range(NCORES)))

    out = np.empty((B, S, H), np.float32)
    for core in range(NCORES):
        b, parity = core // 2, core % 2
        o = np.asarray(res.results[core]["out"], np.float32).reshape(NMYB, 128, H)
        out[b].reshape(32, 128, H)[parity::2] = o
    return out


# revision 25
# speedup vs baseline: 1.4420x; 1.1721x over previous
"""Causal single-head attention (B=4, S=4096, D=1024, H=64) on 8 TRN2 NeuronCores.

Strategy (v3: sequence-parallel, peer k/v computed locally)
-----------------------------------------------------------
Data-parallel over batch (2 cores per batch element); within a pair the q rows
are split by 128-row block parity (even core: natural blocks 0,2,..,30; odd:
1,3,..,31), which load-balances the causal triangle.  Each core loads BOTH
halves of its batch element's activations (bf16, 8 MB) and computes q for its
own blocks plus k/v for ALL 32 blocks locally — no collectives, no exchange:
every dependency in the attention pipeline is a short on-core PE->DVE->PE hop.
Each core then produces COMPLETE attention rows for its own q blocks, so the
softmax denominator is a per-partition scalar and outputs are written directly.

Layouts chosen for the TRN2 cost model (matmul cost = moving columns only):
- All matmuls bf16 (1 col/cycle at any width; f32 weights would cost 4x).
- Projections: my tokens stream through lhsT=[Wq|Wk] (128 wide); peer tokens
  stream through lhsT=[Wk|Wv] — a contiguous slice of the same packed weight
  wall.  My v is projected directly in natural [token, h] layout (x chunk as
  lhsT, Wv moving: 64 cols per block); peer v arrives transposed in the
  [Wk|Wv] pass and is turned natural by a PE transpose (64 cols each).
- kT is [h, kpos] so scores^T tiles [128 k, 128 q] come out k-on-partitions,
  which is exactly the lhsT layout the out matmul wants:
  out_acc[128 q, 65] += lhsT=exp_tile, rhs=v_nat[128 k, 65] — only 65 moving
  cols per (k block, q block).  Col 64 of v_nat is constant 1, so the softmax
  denominator accumulates as output column 64 for free.
- The causal mask is applied ON the tensor engine: an extra accumulating
  matmul lhsT=I, rhs=mask_tile adds -8e9 into the banded scores PSUM.
  Host-computed mask data absorbs the even/odd parity asymmetry (the SPMD
  instruction stream is identical on all 8 cores): slot (0, j) holds my
  parity's block j, slot (1, j) the peer's; for q block i, slot (0, i) is
  always the diagonal (triangular mask) and slot (1, i) is fully masked on
  even cores / fully valid on odd cores, so extents are uniformly 2i+2.
- exp runs on the scalar engine out of PSUM in [128, 8*128] tiles (35 ops
  total) with the 1/8 scale folded in; masked entries underflow to exactly 0,
  matching the reference's -1e9 semantics.  exp outputs bf16; the scalar
  queue carries ONLY exp so it never sits behind a waiting DMA.
- Normalization: out_acc col 64 is the denominator (per-partition scalar) ->
  DVE reciprocal + tensor_scalar_mul, then direct DMA of the final rows.

The host only does layout work plus the fp32->bf16 cast of the inputs
(~5e-3 worst-case relative error, well inside the 2e-2 gate).
"""

import numpy as np
from contextlib import ExitStack

import concourse.bass as bass
import concourse.mybir as mybir
import concourse.tile as tile
from concourse import bacc
from concourse.bass_utils import run_bass_kernel_spmd
from concourse.masks import make_identity

F32 = mybir.dt.float32
BF16 = mybir.dt.bfloat16

B, S, D, H = 4, 4096, 1024, 64
NCORES = 8
NCH = D // 128        # contraction chunks
NMYB = 16             # my q blocks per core
MASK_VAL = -8.0e9     # raw-score mask; exp(0.125*(s+MASK_VAL)) == 0
SCALE = 0.125         # 1/sqrt(H)
GPT = 8               # score/exp groups per PSUM tile ([128, GPT*128])


def build_program(with_cc: bool = True):
    nc = bacc.Bacc(num_devices=NCORES)

    xT = nc.declare_dram_parameter("xT", [D, S], BF16, isOutput=False)
    wall = nc.declare_dram_parameter("wall", [D, 256], BF16, isOutput=False)
    bqk = nc.declare_dram_parameter("bqk", [2 * H, 1], F32, isOutput=False)
    bv = nc.declare_dram_parameter("bv", [1, H], F32, isOutput=False)
    masks = nc.declare_dram_parameter("masks", [2, 128, 128], BF16, isOutput=False)
    ident = nc.declare_dram_parameter("ident", [128, 128], BF16, isOutput=False)
    out = nc.declare_dram_parameter("out", [S // 2, H], F32, isOutput=True)

    xT3 = xT.rearrange("(c p) s -> p c s", p=128)        # [128, 8, 4096]
    wall3 = wall.rearrange("(c p) h -> p c h", p=128)    # [128, 8, 256]
    masks3 = masks.rearrange("m p j -> p m j")           # [128, 2, 128]
    out3 = out.rearrange("(n p) h -> p n h", p=128)      # [128, 16, 64]

    with ExitStack() as ctx:
        tc = ctx.enter_context(tile.TileContext(nc))

        singles = ctx.enter_context(tc.tile_pool(name="singles", bufs=1))

        xt_all = singles.tile([128, NCH, S], BF16)
        wall_sb = singles.tile([128, NCH, 256], BF16)
        bqk_sb = singles.tile([2 * H, 1], F32)
        bv_bc = singles.tile([128, H], F32)
        masks_sb = singles.tile([128, 2, 128], BF16)
        ident_sb = singles.tile([128, 128], BF16)
        ident64 = singles.tile([H, H], F32)
        make_identity(nc, ident64)

        # priority order on the sync queue: tiny params, weights, then the x
        # stream (my/peer piece pairs in consumption order)
        nc.sync.dma_start(out=ident_sb, in_=ident[:, :])
        nc.sync.dma_start(out=bqk_sb, in_=bqk[:, :])
        nc.sync.dma_start(out=masks_sb, in_=masks3)
        bv_b = bass.AP(tensor=bv[:, :].tensor, offset=bv[:, :].offset,
                       ap=[[0, 128], [1, H]])
        nc.sync.dma_start(out=bv_bc, in_=bv_b)
        nc.sync.dma_start(out=wall_sb, in_=wall3)

        def x_dma(lo, n):
            nc.sync.dma_start(
                out=xt_all[:, :, lo : lo + n], in_=xT3[:, :, lo : lo + n]
            )

        # persistent SBUF state; slot (0, j) = my block j, (1, j) = peer's
        qT_sb = singles.tile([H, NMYB, 128], BF16)
        kT_sb = singles.tile([H, 2, NMYB, 128], BF16)
        v_sb = singles.tile([128, 2, NMYB, 65], BF16)
        out_stage = singles.tile([128, NMYB, H], F32)
        part_sb = singles.tile([128, NMYB, H + 1], F32)
        rcp_sb = singles.tile([128, NMYB], F32)
        nc.vector.memset(v_sb[:, :, :, H : H + 1], 1.0)

        pj = ctx.enter_context(tc.tile_pool(name="pj", bufs=2, space="PSUM"))
        ps = ctx.enter_context(tc.tile_pool(name="ps", bufs=2, space="PSUM"))
        pacc = ctx.enter_context(tc.tile_pool(name="pacc", bufs=2, space="PSUM"))
        pexp_pool = ctx.enter_context(tc.tile_pool(name="pexp", bufs=5))
        vpt = ctx.enter_context(tc.tile_pool(name="vpt", bufs=2))

        def proj_mine(p):
            """q/k/v for my token blocks 2p, 2p+1 (xt cols 256p..)."""
            lo = 256 * p
            b0 = 2 * p
            # [q|k] 0:256 | v_nat b0 256:320 | v_nat b0+1 320:384
            pjt = pj.tile([128, 384], F32, tag="pj", name="pjt")
            for ch in range(NCH):
                nc.tensor.matmul(
                    pjt[:, 0:256], lhsT=wall_sb[:, ch, 0 : 2 * H],
                    rhs=xt_all[:, ch, lo : lo + 256],
                    start=(ch == 0), stop=(ch == NCH - 1),
                )
            for b2 in range(2):
                for ch in range(NCH):
                    nc.tensor.matmul(
                        pjt[:, 256 + 64 * b2 : 320 + 64 * b2],
                        lhsT=xt_all[:, ch, lo + 128 * b2 : lo + 128 * b2 + 128],
                        rhs=wall_sb[:, ch, 2 * H : 3 * H],
                        start=(ch == 0), stop=(ch == NCH - 1),
                    )
            nc.vector.tensor_scalar_add(
                qT_sb[:, b0 : b0 + 2, :], pjt[0:H, 0:256], bqk_sb[0:H, :]
            )
            nc.vector.tensor_scalar_add(
                kT_sb[:, 0, b0 : b0 + 2, :], pjt[H : 2 * H, 0:256],
                bqk_sb[H : 2 * H, :],
            )
            nc.vector.tensor_add(v_sb[:, 0, b0, 0:H], pjt[:, 256:320], bv_bc)
            nc.vector.tensor_add(v_sb[:, 0, b0 + 1, 0:H], pjt[:, 320:384], bv_bc)

        def proj_peer(p):
            """k/vT for peer token blocks 2p, 2p+1 (xt cols 2048+256p..)."""
            lo = S // 2 + 256 * p
            b0 = 2 * p
            # [k|vT] 0:256 | v_nat b0 256:320 | v_nat b0+1 320:384
            pjt = pj.tile([128, 384], F32, tag="pj", name="pjt")
            for ch in range(NCH):
                nc.tensor.matmul(
                    pjt[:, 0:256], lhsT=wall_sb[:, ch, H : 3 * H],
                    rhs=xt_all[:, ch, lo : lo + 256],
                    start=(ch == 0), stop=(ch == NCH - 1),
                )
            vpt_t = vpt.tile([H, 2, 128], F32, tag="vpt", name="vpt_t")
            nc.vector.tensor_scalar_add(
                kT_sb[:, 1, b0 : b0 + 2, :], pjt[0:H, 0:256],
                bqk_sb[H : 2 * H, :],
            )
            nc.vector.tensor_copy(vpt_t, pjt[H : 2 * H, 0:256])
            for b2 in range(2):
                nc.tensor.transpose(
                    pjt[:, 256 + 64 * b2 : 320 + 64 * b2], vpt_t[:, b2, :],
                    ident64,
                )
                nc.vector.tensor_add(
                    v_sb[:, 1, b0 + b2, 0:H],
                    pjt[:, 256 + 64 * b2 : 320 + 64 * b2], bv_bc,
                )

        # ---- attention pipeline ----
        cur = {"ps": None, "n": 0, "meta": []}
        deferred = []  # (ps_tile, pexp_tile, meta) awaiting out-matmul emission
        acc_of = {}

        def get_acc(i, r):
            if (i, r) not in acc_of:
                t = pacc.tile([128, 2, 66], F32, tag="pacc", name="acc_t")
                acc_of[(i, r)] = t
                acc_of[(i + 1, r)] = t
            return acc_of[(i, r)]

        def emit_out(batch):
            """Out matmuls (drain/combine/norm) for a completed exp tile."""
            ps_t, px_t, meta = batch
            for g, (i, r, j) in enumerate(meta):
                acc = acc_of[(i, r)]
                nc.tensor.matmul(
                    acc[:, i % 2, 0:65],
                    lhsT=px_t[:, g, :],
                    rhs=v_sb[:, r, j, :],
                    start=(j == 0),
                    stop=(j == i),
                )
                if j != i:
                    continue
                if r == 0:
                    # drain the my-half partial so the PSUM slot recycles
                    nc.vector.tensor_copy(
                        part_sb[:, i, :], acc[:, i % 2, 0:65]
                    )
                    continue
                # combine halves, then normalize by the summed denominator
                nc.vector.tensor_add(
                    part_sb[:, i, :], acc[:, i % 2, 0:65], part_sb[:, i, :]
                )
                if True:
                    nc.vector.reciprocal(
                        rcp_sb[:, i : i + 1], part_sb[:, i, 64:65]
                    )
                    nc.vector.tensor_scalar_mul(
                        out_stage[:, i, :], part_sb[:, i, 0:64],
                        rcp_sb[:, i : i + 1],
                    )
                    # final rows: progressively finer pieces on the sync
                    # queue so the last norm->write->drain tail is short
                    if i == 3 or i == 7 or i == 11:
                        g4 = i // 4
                        nc.gpsimd.dma_start(
                            out=out3[:, 4 * g4 : 4 * g4 + 4, :],
                            in_=out_stage[:, 4 * g4 : 4 * g4 + 4, :],
                        )
                    elif i == 13:
                        nc.sync.dma_start(
                            out=out3[:, 12:14, :], in_=out_stage[:, 12:14, :]
                        )
                    elif i >= 14:
                        nc.sync.dma_start(
                            out=out3[:, i : i + 1, :],
                            in_=out_stage[:, i : i + 1, :],
                        )

        def flush_tile():
            """Close the current scores tile: exp it, queue its out matmuls."""
            if cur["n"] == 0:
                return
            ps_t, meta = cur["ps"], cur["meta"]
            px_t = pexp_pool.tile([128, GPT, 128], BF16, tag="px", name="px_t")
            n = cur["n"]
            nc.scalar.activation(
                px_t[:, 0:n, :], ps_t[:, 0:n, :],
                mybir.ActivationFunctionType.Exp, scale=SCALE,
            )
            deferred.append((ps_t, px_t, list(meta)))
            cur["ps"], cur["n"], cur["meta"] = None, 0, []
            if len(deferred) > 3:
                emit_out(deferred.pop(0))

        FILL = 0

        def group(i, r, j):
            """Scores (+mask) for q block i vs k slot (r, j)."""
            if cur["ps"] is None:
                cur["ps"] = ps.tile([128, GPT, 128], F32, tag="ps", name="ps_t")
                # filler matmuls: keep the tensor engine p-state ramped while
                # it waits for exp to free the ring; overwritten by start=True
                for f in range(FILL):
                    nc.tensor.matmul(
                        cur["ps"][:, 2 * f : 2 * f + 2, :],
                        lhsT=ident_sb, rhs=masks_sb[:, :, :],
                        start=True, stop=True,
                    )
            g = cur["n"]
            ps_t = cur["ps"]
            masked = j == i
            nc.tensor.matmul(
                ps_t[:, g, :], lhsT=kT_sb[:, r, j, :], rhs=qT_sb[:, i, :],
                start=True, stop=not masked,
            )
            if masked:
                nc.tensor.matmul(
                    ps_t[:, g, :], lhsT=ident_sb, rhs=masks_sb[:, r, :],
                    start=False, stop=True,
                )
            cur["meta"].append((i, r, j))
            cur["n"] += 1
            if cur["n"] == GPT:
                flush_tile()

        def wave_r(w, r):
            """One rank-half of attention for my q blocks 2w, 2w+1."""
            get_acc(2 * w, r)
            for i in (2 * w, 2 * w + 1):
                for j in range(i + 1):
                    group(i, r, j)

        # ---- pipelined emission ----
        x_dma(0, 256)           # my blocks 0-1
        x_dma(256, 256)         # my blocks 2-3
        x_dma(S // 2, 256)      # peer blocks 0-1

        # PE warmup: dependency-free matmuls so the tensor engine is at full
        # p-state when proj 0's data lands
        wps = ps.tile([128, GPT, 128], F32, tag="ps", name="wps")
        for wi in range(32):
            nc.tensor.matmul(
                wps[:, wi % GPT, :], lhsT=ident_sb, rhs=ident_sb,
                start=True, stop=True,
            )

        for p in range(8):
            proj_mine(p)
            wave_r(p, 0)
            if p == 0:
                flush_tile()  # release the first groups early
            if p < 6:
                x_dma(256 * (p + 2), 256)
            proj_peer(p)
            wave_r(p, 1)
            if p < 7:
                x_dma(S // 2 + 256 * (p + 1), 256)
        flush_tile()
        while deferred:
            emit_out(deferred.pop(0))

    nc.finalize()
    return nc


_PROGRAM_CACHE = {}


def _get_program():
    if "prog" not in _PROGRAM_CACHE:
        _PROGRAM_CACHE["prog"] = build_program()
    return _PROGRAM_CACHE["prog"]


def _bf16(a):
    import ml_dtypes
    return np.asarray(a).astype(ml_dtypes.bfloat16)


def _make_masks(parity: int) -> np.ndarray:
    """[2, 128, 128] additive mask tiles for the j == i k slot.

    Slot (0, i) is my own block i = the diagonal (strict lower triangle of
    scores^T masked: k row p > q col j).  Slot (1, i) is the peer's block i:
    natural 2i+1 > 2i for even cores (fully masked), natural 2i < 2i+1 for
    odd cores (fully valid).
    """
    p = np.arange(128)[:, None]
    j = np.arange(128)[None, :]
    tri = np.where(p > j, MASK_VAL, 0.0).astype(np.float32)
    full = np.full((128, 128), MASK_VAL, np.float32)
    zero = np.zeros((128, 128), np.float32)
    m = np.stack([tri, full if parity == 0 else zero])
    return _bf16(m)


def kernel(x, Wq, bq, Wk, bk, Wv, bv):
    x = np.asarray(x, dtype=np.float32)
    wall = np.zeros((D, 256), np.float32)
    wall[:, 0:H] = np.asarray(Wq)
    wall[:, H : 2 * H] = np.asarray(Wk)
    wall[:, 2 * H : 3 * H] = np.asarray(Wv)
    wall = _bf16(wall)
    bqk = np.concatenate(
        [np.asarray(bq), np.asarray(bk)]
    ).astype(np.float32).reshape(2 * H, 1)
    bv_ = np.asarray(bv, dtype=np.float32).reshape(1, H)
    ident = _bf16(np.eye(128, dtype=np.float32))

    nc = _get_program()

    in_maps = []
    for core in range(NCORES):
        b, parity = core // 2, core % 2
        xb = x[b].T.reshape(D, 32, 128)
        mine = xb[:, parity::2, :].reshape(D, S // 2)
        peer = xb[:, 1 - parity :: 2, :].reshape(D, S // 2)
        xTp = np.ascontiguousarray(np.concatenate([mine, peer], axis=1))
        in_maps.append(
            {
                "xT": _bf16(xTp),
                "wall": wall,
                "bqk": bqk,
                "bv": bv_,
                "masks": _make_masks(parity),
                "ident": ident,
            }
        )

    res = run_bass_kernel_spmd(nc, in_maps, list(range(NCORES)))

    out = np.empty((B, S, H), np.float32)
    for core in range(NCORES):
        b, parity = core // 2, core % 2
        o = np.asarray(res.results[core]["out"], np.float32).reshape(NMYB, 128, H)
        out[b].reshape(32, 128, H)[parity::2] = o
    return out


# revision 32
# speedup vs baseline: 1.4439x; 1.0013x over previous
"""Causal single-head attention (B=4, S=4096, D=1024, H=64) on 8 TRN2 NeuronCores.

Strategy (v3: sequence-parallel, peer k/v computed locally)
-----------------------------------------------------------
Data-parallel over batch (2 cores per batch element); within a pair the q rows
are split by 128-row block parity (even core: natural blocks 0,2,..,30; odd:
1,3,..,31), which load-balances the causal triangle.  Each core loads BOTH
halves of its batch element's activations (bf16, 8 MB) and computes q for its
own blocks plus k/v for ALL 32 blocks locally — no collectives, no exchange:
every dependency in the attention pipeline is a short on-core PE->DVE->PE hop.
Each core then produces COMPLETE attention rows for its own q blocks, so the
softmax denominator is a per-partition scalar and outputs are written directly.

Layouts chosen for the TRN2 cost model (matmul cost = moving columns only):
- All matmuls bf16 (1 col/cycle at any width; f32 weights would cost 4x).
- Projections: my tokens stream through lhsT=[Wq|Wk] (128 wide); peer tokens
  stream through lhsT=[Wk|Wv] — a contiguous slice of the same packed weight
  wall.  My v is projected directly in natural [token, h] layout (x chunk as
  lhsT, Wv moving: 64 cols per block); peer v arrives transposed in the
  [Wk|Wv] pass and is turned natural by a PE transpose (64 cols each).
- kT is [h, kpos] so scores^T tiles [128 k, 128 q] come out k-on-partitions,
  which is exactly the lhsT layout the out matmul wants:
  out_acc[128 q, 65] += lhsT=exp_tile, rhs=v_nat[128 k, 65] — only 65 moving
  cols per (k block, q block).  Col 64 of v_nat is constant 1, so the softmax
  denominator accumulates as output column 64 for free.
- The causal mask is applied ON the tensor engine: an extra accumulating
  matmul lhsT=I, rhs=mask_tile adds -8e9 into the banded scores PSUM.
  Host-computed mask data absorbs the even/odd parity asymmetry (the SPMD
  instruction stream is identical on all 8 cores): slot (0, j) holds my
  parity's block j, slot (1, j) the peer's; for q block i, slot (0, i) is
  always the diagonal (triangular mask) and slot (1, i) is fully masked on
  even cores / fully valid on odd cores, so extents are uniformly 2i+2.
- exp runs on the scalar engine out of PSUM in [128, 8*128] tiles (35 ops
  total) with the 1/8 scale folded in; masked entries underflow to exactly 0,
  matching the reference's -1e9 semantics.  exp outputs bf16; the scalar
  queue carries ONLY exp so it never sits behind a waiting DMA.
- Normalization: out_acc col 64 is the denominator (per-partition scalar) ->
  DVE reciprocal + tensor_scalar_mul, then direct DMA of the final rows.

The host only does layout work plus the fp32->bf16 cast of the inputs
(~5e-3 worst-case relative error, well inside the 2e-2 gate).
"""

import numpy as np
from contextlib import ExitStack

import concourse.bass as bass
import concourse.mybir as mybir
import concourse.tile as tile
from concourse import bacc
from concourse.bass_utils import run_bass_kernel_spmd
from concourse.masks import make_identity

F32 = mybir.dt.float32
BF16 = mybir.dt.bfloat16

B, S, D, H = 4, 4096, 1024, 64
NCORES = 8
NCH = D // 128        # contraction chunks
NMYB = 16             # my q blocks per core
MASK_VAL = -8.0e9     # raw-score mask; exp(0.125*(s+MASK_VAL)) == 0
SCALE = 0.125         # 1/sqrt(H)
GPT = 8               # score/exp groups per PSUM tile ([128, GPT*128])


def build_program(with_cc: bool = True):
    nc = bacc.Bacc(num_devices=NCORES)

    xT = nc.declare_dram_parameter("xT", [D, S], BF16, isOutput=False)
    wall = nc.declare_dram_parameter("wall", [D, 256], BF16, isOutput=False)
    bqk = nc.declare_dram_parameter("bqk", [2 * H, 1], F32, isOutput=False)
    bv = nc.declare_dram_parameter("bv", [1, H], F32, isOutput=False)
    masks = nc.declare_dram_parameter("masks", [2, 128, 128], BF16, isOutput=False)
    ident = nc.declare_dram_parameter("ident", [128, 128], BF16, isOutput=False)
    out = nc.declare_dram_parameter("out", [S // 2, H], F32, isOutput=True)

    xT3 = xT.rearrange("(c p) s -> p c s", p=128)        # [128, 8, 4096]
    wall3 = wall.rearrange("(c p) h -> p c h", p=128)    # [128, 8, 256]
    masks3 = masks.rearrange("m p j -> p m j")           # [128, 2, 128]
    out3 = out.rearrange("(n p) h -> p n h", p=128)      # [128, 16, 64]

    with ExitStack() as ctx:
        tc = ctx.enter_context(tile.TileContext(nc))

        singles = ctx.enter_context(tc.tile_pool(name="singles", bufs=1))

        xt_all = singles.tile([128, NCH, S], BF16)
        wall_sb = singles.tile([128, NCH, 256], BF16)
        bqk_sb = singles.tile([2 * H, 1], F32)
        bv_bc = singles.tile([128, H], F32)
        masks_sb = singles.tile([128, 2, 128], BF16)
        ident_sb = singles.tile([128, 128], BF16)
        ident64 = singles.tile([H, H], F32)
        make_identity(nc, ident64)

        # priority order on the sync queue: tiny params, weights, then the x
        # stream (my/peer piece pairs in consumption order)
        nc.sync.dma_start(out=ident_sb, in_=ident[:, :])
        nc.sync.dma_start(out=bqk_sb, in_=bqk[:, :])
        nc.sync.dma_start(out=masks_sb, in_=masks3)
        bv_b = bass.AP(tensor=bv[:, :].tensor, offset=bv[:, :].offset,
                       ap=[[0, 128], [1, H]])
        nc.sync.dma_start(out=bv_bc, in_=bv_b)
        nc.sync.dma_start(out=wall_sb, in_=wall3)

        def x_dma(lo, n):
            nc.sync.dma_start(
                out=xt_all[:, :, lo : lo + n], in_=xT3[:, :, lo : lo + n]
            )

        # persistent SBUF state; slot (0, j) = my block j, (1, j) = peer's
        qT_sb = singles.tile([H, NMYB, 128], BF16)
        kT_sb = singles.tile([H, 2, NMYB, 128], BF16)
        v_sb = singles.tile([128, 2, NMYB, 65], BF16)
        out_stage = singles.tile([128, NMYB, H], F32)
        part_sb = singles.tile([128, NMYB, H + 1], F32)
        rcp_sb = singles.tile([128, NMYB], F32)
        nc.vector.memset(v_sb[:, :, :, H : H + 1], 1.0)

        pj = ctx.enter_context(tc.tile_pool(name="pj", bufs=2, space="PSUM"))
        ps = ctx.enter_context(tc.tile_pool(name="ps", bufs=2, space="PSUM"))
        pacc = ctx.enter_context(tc.tile_pool(name="pacc", bufs=2, space="PSUM"))
        pexp_pool = ctx.enter_context(tc.tile_pool(name="pexp", bufs=6))
        vpt = ctx.enter_context(tc.tile_pool(name="vpt", bufs=2))

        def proj_mine(p):
            """q/k/v for my token blocks 2p, 2p+1 (xt cols 256p..)."""
            lo = 256 * p
            b0 = 2 * p
            # [q|k] 0:256 | v_nat b0 256:320 | v_nat b0+1 320:384
            pjt = pj.tile([128, 384], F32, tag="pj", name="pjt")
            for ch in range(NCH):
                nc.tensor.matmul(
                    pjt[:, 0:256], lhsT=wall_sb[:, ch, 0 : 2 * H],
                    rhs=xt_all[:, ch, lo : lo + 256],
                    start=(ch == 0), stop=(ch == NCH - 1),
                )
            for b2 in range(2):
                for ch in range(NCH):
                    nc.tensor.matmul(
                        pjt[:, 256 + 64 * b2 : 320 + 64 * b2],
                        lhsT=xt_all[:, ch, lo + 128 * b2 : lo + 128 * b2 + 128],
                        rhs=wall_sb[:, ch, 2 * H : 3 * H],
                        start=(ch == 0), stop=(ch == NCH - 1),
                    )
            nc.vector.tensor_scalar_add(
                qT_sb[:, b0 : b0 + 2, :], pjt[0:H, 0:256], bqk_sb[0:H, :]
            )
            nc.vector.tensor_scalar_add(
                kT_sb[:, 0, b0 : b0 + 2, :], pjt[H : 2 * H, 0:256],
                bqk_sb[H : 2 * H, :],
            )
            nc.vector.tensor_add(v_sb[:, 0, b0, 0:H], pjt[:, 256:320], bv_bc)
            nc.vector.tensor_add(v_sb[:, 0, b0 + 1, 0:H], pjt[:, 320:384], bv_bc)

        def proj_peer(p):
            """k/vT for peer token blocks 2p, 2p+1 (xt cols 2048+256p..)."""
            lo = S // 2 + 256 * p
            b0 = 2 * p
            # [k|vT] 0:256 | v_nat b0 256:320 | v_nat b0+1 320:384
            pjt = pj.tile([128, 384], F32, tag="pj", name="pjt")
            for ch in range(NCH):
                nc.tensor.matmul(
                    pjt[:, 0:256], lhsT=wall_sb[:, ch, H : 3 * H],
                    rhs=xt_all[:, ch, lo : lo + 256],
                    start=(ch == 0), stop=(ch == NCH - 1),
                )
            vpt_t = vpt.tile([H, 2, 128], F32, tag="vpt", name="vpt_t")
            nc.vector.tensor_scalar_add(
                kT_sb[:, 1, b0 : b0 + 2, :], pjt[0:H, 0:256],
                bqk_sb[H : 2 * H, :],
            )
            nc.vector.tensor_copy(vpt_t, pjt[H : 2 * H, 0:256])
            for b2 in range(2):
                nc.tensor.transpose(
                    pjt[:, 256 + 64 * b2 : 320 + 64 * b2], vpt_t[:, b2, :],
                    ident64,
                )
                nc.vector.tensor_add(
                    v_sb[:, 1, b0 + b2, 0:H],
                    pjt[:, 256 + 64 * b2 : 320 + 64 * b2], bv_bc,
                )

        # ---- attention pipeline ----
        cur = {"ps": None, "n": 0, "meta": []}
        deferred = []  # (ps_tile, pexp_tile, meta) awaiting out-matmul emission
        acc_of = {}

        def get_acc(i, r):
            if (i, r) not in acc_of:
                t = pacc.tile([128, 2, 66], F32, tag="pacc", name="acc_t")
                acc_of[(i, r)] = t
                acc_of[(i + 1, r)] = t
            return acc_of[(i, r)]

        def emit_out(batch):
            """Out matmuls (drain/combine/norm) for a completed exp tile."""
            ps_t, px_t, meta = batch
            for g, (i, r, j) in enumerate(meta):
                acc = acc_of[(i, r)]
                nc.tensor.matmul(
                    acc[:, i % 2, 0:65],
                    lhsT=px_t[:, g, :],
                    rhs=v_sb[:, r, j, :],
                    start=(j == 0),
                    stop=(j == i),
                )
                if j != i:
                    continue
                if r == 0:
                    # drain the my-half partial so the PSUM slot recycles
                    nc.vector.tensor_copy(
                        part_sb[:, i, :], acc[:, i % 2, 0:65]
                    )
                    continue
                # combine halves, then normalize by the summed denominator
                nc.vector.tensor_add(
                    part_sb[:, i, :], acc[:, i % 2, 0:65], part_sb[:, i, :]
                )
                if True:
                    nc.vector.reciprocal(
                        rcp_sb[:, i : i + 1], part_sb[:, i, 64:65]
                    )
                    nc.vector.tensor_scalar_mul(
                        out_stage[:, i, :], part_sb[:, i, 0:64],
                        rcp_sb[:, i : i + 1],
                    )
                    # final rows: progressively finer pieces on the sync
                    # queue so the last norm->write->drain tail is short
                    if i == 3 or i == 7 or i == 11:
                        g4 = i // 4
                        nc.gpsimd.dma_start(
                            out=out3[:, 4 * g4 : 4 * g4 + 4, :],
                            in_=out_stage[:, 4 * g4 : 4 * g4 + 4, :],
                        )
                    elif i == 13:
                        nc.sync.dma_start(
                            out=out3[:, 12:14, :], in_=out_stage[:, 12:14, :]
                        )
                    elif i >= 14:
                        nc.sync.dma_start(
                            out=out3[:, i : i + 1, :],
                            in_=out_stage[:, i : i + 1, :],
                        )

        def flush_tile():
            """Close the current scores tile: exp it, queue its out matmuls."""
            if cur["n"] == 0:
                return
            ps_t, meta = cur["ps"], cur["meta"]
            px_t = pexp_pool.tile([128, GPT, 128], BF16, tag="px", name="px_t")
            n = cur["n"]
            nc.scalar.activation(
                px_t[:, 0:n, :], ps_t[:, 0:n, :],
                mybir.ActivationFunctionType.Exp, scale=SCALE,
            )
            deferred.append((ps_t, px_t, list(meta)))
            cur["ps"], cur["n"], cur["meta"] = None, 0, []
            if len(deferred) > 3:
                emit_out(deferred.pop(0))

        FILL = 0

        def group(i, r, j):
            """Scores (+mask) for q block i vs k slot (r, j)."""
            if cur["ps"] is None:
                cur["ps"] = ps.tile([128, GPT, 128], F32, tag="ps", name="ps_t")
                # filler matmuls: keep the tensor engine p-state ramped while
                # it waits for exp to free the ring; overwritten by start=True
                for f in range(FILL):
                    nc.tensor.matmul(
                        cur["ps"][:, 2 * f : 2 * f + 2, :],
                        lhsT=ident_sb, rhs=masks_sb[:, :, :],
                        start=True, stop=True,
                    )
            g = cur["n"]
            ps_t = cur["ps"]
            masked = j == i
            nc.tensor.matmul(
                ps_t[:, g, :], lhsT=kT_sb[:, r, j, :], rhs=qT_sb[:, i, :],
                start=True, stop=not masked,
            )
            if masked:
                nc.tensor.matmul(
                    ps_t[:, g, :], lhsT=ident_sb, rhs=masks_sb[:, r, :],
                    start=False, stop=True,
                )
            cur["meta"].append((i, r, j))
            cur["n"] += 1
            if cur["n"] == GPT:
                flush_tile()

        def wave_r(w, r):
            """One rank-half of attention for my q blocks 2w, 2w+1."""
            get_acc(2 * w, r)
            for i in (2 * w, 2 * w + 1):
                for j in range(i + 1):
                    group(i, r, j)

        # ---- pipelined emission ----
        x_dma(0, 256)           # my blocks 0-1
        x_dma(256, 256)         # my blocks 2-3
        x_dma(S // 2, 256)      # peer blocks 0-1

        # PE warmup: dependency-free matmuls so the tensor engine is at full
        # p-state when proj 0's data lands
        wps = ps.tile([128, GPT, 128], F32, tag="ps", name="wps")
        for wi in range(32):
            nc.tensor.matmul(
                wps[:, wi % GPT, :], lhsT=ident_sb, rhs=ident_sb,
                start=True, stop=True,
            )

        for p in range(8):
            proj_mine(p)
            wave_r(p, 0)
            if p == 0:
                flush_tile()  # release the first groups early
            if p < 6:
                x_dma(256 * (p + 2), 256)
            proj_peer(p)
            wave_r(p, 1)
            if p < 7:
                x_dma(S // 2 + 256 * (p + 1), 256)
        flush_tile()
        while deferred:
            emit_out(deferred.pop(0))

    nc.finalize()
    return nc


_PROGRAM_CACHE = {}


def _get_program():
    if "prog" not in _PROGRAM_CACHE:
        _PROGRAM_CACHE["prog"] = build_program()
    return _PROGRAM_CACHE["prog"]


def _bf16(a):
    import ml_dtypes
    return np.asarray(a).astype(ml_dtypes.bfloat16)


def _make_masks(parity: int) -> np.ndarray:
    """[2, 128, 128] additive mask tiles for the j == i k slot.

    Slot (0, i) is my own block i = the diagonal (strict lower triangle of
    scores^T masked: k row p > q col j).  Slot (1, i) is the peer's block i:
    natural 2i+1 > 2i for even cores (fully masked), natural 2i < 2i+1 for
    odd cores (fully valid).
    """
    p = np.arange(128)[:, None]
    j = np.arange(128)[None, :]
    tri = np.where(p > j, MASK_VAL, 0.0).astype(np.float32)
    full = np.full((128, 128), MASK_VAL, np.float32)
    zero = np.zeros((128, 128), np.float32)
    m = np.stack([tri, full if parity == 0 else zero])
    return _bf16(m)


def kernel(x, Wq, bq, Wk, bk, Wv, bv):
    x = np.asarray(x, dtype=np.float32)
    wall = np.zeros((D, 256), np.float32)
    wall[:, 0:H] = np.asarray(Wq)
    wall[:, H : 2 * H] = np.asarray(Wk)
    wall[:, 2 * H : 3 * H] = np.asarray(Wv)
    wall = _bf16(wall)
    bqk = np.concatenate(
        [np.asarray(bq), np.asarray(bk)]
    ).astype(np.float32).reshape(2 * H, 1)
    bv_ = np.asarray(bv, dtype=np.float32).reshape(1, H)
    ident = _bf16(np.eye(128, dtype=np.float32))

    nc = _get_program()

    in_maps = []
    for core in range(NCORES):
        b, parity = core // 2, core % 2
        xb = x[b].T.reshape(D, 32, 128)
        mine = xb[:, parity::2, :].reshape(D, S // 2)
        peer = xb[:, 1 - parity :: 2, :].reshape(D, S // 2)
        xTp = np.ascontiguousarray(np.concatenate([mine, peer], axis=1))
        in_maps.append(
            {
                "xT": _bf16(xTp),
                "wall": wall,
                "bqk": bqk,
                "bv": bv_,
                "masks": _make_masks(parity),
                "ident": ident,
            }
        )

    res = run_bass_kernel_spmd(nc, in_maps, list(range(NCORES)))

    out = np.empty((B, S, H), np.float32)
    for core in range(NCORES):
        b, parity = core // 2, core % 2
        o = np.asarray(res.results[core]["out"], np.float32).reshape(NMYB, 128, H)
        out[b].reshape(32, 128, H)[parity::2] = o
    return out


# revision 33
# speedup vs baseline: 1.4865x; 1.0295x over previous
"""Causal single-head attention (B=4, S=4096, D=1024, H=64) on 8 TRN2 NeuronCores.

Strategy (v3: sequence-parallel, peer k/v computed locally)
-----------------------------------------------------------
Data-parallel over batch (2 cores per batch element); within a pair the q rows
are split by 128-row block parity (even core: natural blocks 0,2,..,30; odd:
1,3,..,31), which load-balances the causal triangle.  Each core loads BOTH
halves of its batch element's activations (bf16, 8 MB) and computes q for its
own blocks plus k/v for ALL 32 blocks locally — no collectives, no exchange:
every dependency in the attention pipeline is a short on-core PE->DVE->PE hop.
Each core then produces COMPLETE attention rows for its own q blocks, so the
softmax denominator is a per-partition scalar and outputs are written directly.

Layouts chosen for the TRN2 cost model (matmul cost = moving columns only):
- All matmuls bf16 (1 col/cycle at any width; f32 weights would cost 4x).
- Projections: my tokens stream through lhsT=[Wq|Wk] (128 wide); peer tokens
  stream through lhsT=[Wk|Wv] — a contiguous slice of the same packed weight
  wall.  My v is projected directly in natural [token, h] layout (x chunk as
  lhsT, Wv moving: 64 cols per block); peer v arrives transposed in the
  [Wk|Wv] pass and is turned natural by a PE transpose (64 cols each).
- kT is [h, kpos] so scores^T tiles [128 k, 128 q] come out k-on-partitions,
  which is exactly the lhsT layout the out matmul wants:
  out_acc[128 q, 65] += lhsT=exp_tile, rhs=v_nat[128 k, 65] — only 65 moving
  cols per (k block, q block).  Col 64 of v_nat is constant 1, so the softmax
  denominator accumulates as output column 64 for free.
- The causal mask is applied ON the tensor engine: an extra accumulating
  matmul lhsT=I, rhs=mask_tile adds -8e9 into the banded scores PSUM.
  Host-computed mask data absorbs the even/odd parity asymmetry (the SPMD
  instruction stream is identical on all 8 cores): slot (0, j) holds my
  parity's block j, slot (1, j) the peer's; for q block i, slot (0, i) is
  always the diagonal (triangular mask) and slot (1, i) is fully masked on
  even cores / fully valid on odd cores, so extents are uniformly 2i+2.
- exp runs on the scalar engine out of PSUM in [128, 8*128] tiles (35 ops
  total) with the 1/8 scale folded in; masked entries underflow to exactly 0,
  matching the reference's -1e9 semantics.  exp outputs bf16; the scalar
  queue carries ONLY exp so it never sits behind a waiting DMA.
- Normalization: out_acc col 64 is the denominator (per-partition scalar) ->
  DVE reciprocal + tensor_scalar_mul, then direct DMA of the final rows.

The host only does layout work plus the fp32->bf16 cast of the inputs
(~5e-3 worst-case relative error, well inside the 2e-2 gate).
"""

import numpy as np
from contextlib import ExitStack

import concourse.bass as bass
import concourse.mybir as mybir
import concourse.tile as tile
from concourse import bacc
from concourse.bass_utils import run_bass_kernel_spmd
from concourse.masks import make_identity

F32 = mybir.dt.float32
BF16 = mybir.dt.bfloat16

B, S, D, H = 4, 4096, 1024, 64
NCORES = 8
NCH = D // 128        # contraction chunks
NMYB = 16             # my q blocks per core
MASK_VAL = -8.0e9     # raw-score mask; exp(0.125*(s+MASK_VAL)) == 0
SCALE = 0.125         # 1/sqrt(H)
GPT = 8               # score/exp groups per PSUM tile ([128, GPT*128])


def build_program(with_cc: bool = True):
    nc = bacc.Bacc(num_devices=NCORES)

    xT = nc.declare_dram_parameter("xT", [D, S], BF16, isOutput=False)
    wall = nc.declare_dram_parameter("wall", [D, 256], BF16, isOutput=False)
    bqk = nc.declare_dram_parameter("bqk", [2 * H, 1], F32, isOutput=False)
    bv = nc.declare_dram_parameter("bv", [1, H], F32, isOutput=False)
    masks = nc.declare_dram_parameter("masks", [2, 128, 128], BF16, isOutput=False)
    ident = nc.declare_dram_parameter("ident", [128, 128], BF16, isOutput=False)
    out = nc.declare_dram_parameter("out", [S // 2, H], F32, isOutput=True)

    xT3 = xT.rearrange("(c p) s -> p c s", p=128)        # [128, 8, 4096]
    wall3 = wall.rearrange("(c p) h -> p c h", p=128)    # [128, 8, 256]
    masks3 = masks.rearrange("m p j -> p m j")           # [128, 2, 128]
    out3 = out.rearrange("(n p) h -> p n h", p=128)      # [128, 16, 64]

    with ExitStack() as ctx:
        tc = ctx.enter_context(tile.TileContext(nc))

        singles = ctx.enter_context(tc.tile_pool(name="singles", bufs=1))

        xt_all = singles.tile([128, NCH, S], BF16)
        wall_sb = singles.tile([128, NCH, 256], BF16)
        bqk_sb = singles.tile([2 * H, 1], F32)
        bv_bc = singles.tile([128, H], F32)
        masks_sb = singles.tile([128, 2, 128], BF16)
        ident_sb = singles.tile([128, 128], BF16)
        ident64 = singles.tile([H, H], F32)
        make_identity(nc, ident64)

        # priority order on the sync queue: tiny params, weights, then the x
        # stream (my/peer piece pairs in consumption order)
        nc.sync.dma_start(out=ident_sb, in_=ident[:, :])
        nc.sync.dma_start(out=bqk_sb, in_=bqk[:, :])
        nc.sync.dma_start(out=masks_sb, in_=masks3)
        bv_b = bass.AP(tensor=bv[:, :].tensor, offset=bv[:, :].offset,
                       ap=[[0, 128], [1, H]])
        nc.sync.dma_start(out=bv_bc, in_=bv_b)
        nc.sync.dma_start(out=wall_sb, in_=wall3)

        def x_dma(lo, n):
            nc.sync.dma_start(
                out=xt_all[:, :, lo : lo + n], in_=xT3[:, :, lo : lo + n]
            )

        # persistent SBUF state; slot (0, j) = my block j, (1, j) = peer's
        qT_sb = singles.tile([H, NMYB, 128], BF16)
        kT_sb = singles.tile([H, 2, NMYB, 128], BF16)
        v_sb = singles.tile([128, 2, NMYB, 65], BF16)
        out_stage = singles.tile([128, NMYB, H], F32)
        part_sb = singles.tile([128, NMYB, H + 1], F32)
        rcp_sb = singles.tile([128, NMYB], F32)
        nc.vector.memset(v_sb[:, :, :, H : H + 1], 1.0)

        pj = ctx.enter_context(tc.tile_pool(name="pj", bufs=2, space="PSUM"))
        ps = ctx.enter_context(tc.tile_pool(name="ps", bufs=2, space="PSUM"))
        pacc = ctx.enter_context(tc.tile_pool(name="pacc", bufs=2, space="PSUM"))
        pexp_pool = ctx.enter_context(tc.tile_pool(name="pexp", bufs=6))
        vpt = ctx.enter_context(tc.tile_pool(name="vpt", bufs=2))

        def proj_mine(p):
            """q/k/v for my token blocks 2p, 2p+1 (xt cols 256p..)."""
            lo = 256 * p
            b0 = 2 * p
            # [q|k] 0:256 | v_nat b0 256:320 | v_nat b0+1 320:384
            pjt = pj.tile([128, 384], F32, tag="pj", name="pjt")
            for ch in range(NCH):
                nc.tensor.matmul(
                    pjt[:, 0:256], lhsT=wall_sb[:, ch, 0 : 2 * H],
                    rhs=xt_all[:, ch, lo : lo + 256],
                    start=(ch == 0), stop=(ch == NCH - 1),
                )
            for b2 in range(2):
                for ch in range(NCH):
                    nc.tensor.matmul(
                        pjt[:, 256 + 64 * b2 : 320 + 64 * b2],
                        lhsT=xt_all[:, ch, lo + 128 * b2 : lo + 128 * b2 + 128],
                        rhs=wall_sb[:, ch, 2 * H : 3 * H],
                        start=(ch == 0), stop=(ch == NCH - 1),
                    )
            nc.vector.tensor_scalar_add(
                qT_sb[:, b0 : b0 + 2, :], pjt[0:H, 0:256], bqk_sb[0:H, :]
            )
            nc.vector.tensor_scalar_add(
                kT_sb[:, 0, b0 : b0 + 2, :], pjt[H : 2 * H, 0:256],
                bqk_sb[H : 2 * H, :],
            )
            nc.vector.tensor_add(v_sb[:, 0, b0, 0:H], pjt[:, 256:320], bv_bc)
            nc.vector.tensor_add(v_sb[:, 0, b0 + 1, 0:H], pjt[:, 320:384], bv_bc)

        def proj_peer(p):
            """k/vT for peer token blocks 2p, 2p+1 (xt cols 2048+256p..)."""
            lo = S // 2 + 256 * p
            b0 = 2 * p
            # [k|vT] 0:256 | v_nat b0 256:320 | v_nat b0+1 320:384
            pjt = pj.tile([128, 384], F32, tag="pj", name="pjt")
            for ch in range(NCH):
                nc.tensor.matmul(
                    pjt[:, 0:256], lhsT=wall_sb[:, ch, H : 3 * H],
                    rhs=xt_all[:, ch, lo : lo + 256],
                    start=(ch == 0), stop=(ch == NCH - 1),
                )
            vpt_t = vpt.tile([H, 2, 128], F32, tag="vpt", name="vpt_t")
            nc.vector.tensor_scalar_add(
                kT_sb[:, 1, b0 : b0 + 2, :], pjt[0:H, 0:256],
                bqk_sb[H : 2 * H, :],
            )
            nc.vector.tensor_copy(vpt_t, pjt[H : 2 * H, 0:256])
            for b2 in range(2):
                nc.tensor.transpose(
                    pjt[:, 256 + 64 * b2 : 320 + 64 * b2], vpt_t[:, b2, :],
                    ident64,
                )
                nc.vector.tensor_add(
                    v_sb[:, 1, b0 + b2, 0:H],
                    pjt[:, 256 + 64 * b2 : 320 + 64 * b2], bv_bc,
                )

        # ---- attention pipeline ----
        cur = {"ps": None, "n": 0, "meta": []}
        deferred = []  # (ps_tile, pexp_tile, meta) awaiting out-matmul emission
        acc_of = {}

        def get_acc(i, r):
            if (i, r) not in acc_of:
                t = pacc.tile([128, 2, 66], F32, tag="pacc", name="acc_t")
                acc_of[(i, r)] = t
                acc_of[(i + 1, r)] = t
            return acc_of[(i, r)]

        def emit_out(batch):
            """Out matmuls (drain/combine/norm) for a completed exp tile."""
            ps_t, px_t, meta = batch
            for g, (i, r, j) in enumerate(meta):
                acc = acc_of[(i, r)]
                nc.tensor.matmul(
                    acc[:, i % 2, 0:65],
                    lhsT=px_t[:, g, :],
                    rhs=v_sb[:, r, j, :],
                    start=(j == 0),
                    stop=(j == i),
                )
                if j != i:
                    continue
                if r == 0:
                    # drain the my-half partial so the PSUM slot recycles
                    nc.vector.tensor_copy(
                        part_sb[:, i, :], acc[:, i % 2, 0:65]
                    )
                    continue
                # combine halves, then normalize by the summed denominator
                nc.vector.tensor_add(
                    part_sb[:, i, :], acc[:, i % 2, 0:65], part_sb[:, i, :]
                )
                if True:
                    nc.vector.reciprocal(
                        rcp_sb[:, i : i + 1], part_sb[:, i, 64:65]
                    )
                    nc.vector.tensor_scalar_mul(
                        out_stage[:, i, :], part_sb[:, i, 0:64],
                        rcp_sb[:, i : i + 1],
                    )
                    # final rows: progressively finer pieces on the sync
                    # queue so the last norm->write->drain tail is short
                    if i == 3 or i == 7 or i == 11:
                        g4 = i // 4
                        nc.gpsimd.dma_start(
                            out=out3[:, 4 * g4 : 4 * g4 + 4, :],
                            in_=out_stage[:, 4 * g4 : 4 * g4 + 4, :],
                        )
                    elif i == 13:
                        nc.sync.dma_start(
                            out=out3[:, 12:14, :], in_=out_stage[:, 12:14, :]
                        )
                    elif i >= 14:
                        nc.sync.dma_start(
                            out=out3[:, i : i + 1, :],
                            in_=out_stage[:, i : i + 1, :],
                        )

        def flush_tile():
            """Close the current scores tile: exp it, queue its out matmuls."""
            if cur["n"] == 0:
                return
            ps_t, meta = cur["ps"], cur["meta"]
            px_t = pexp_pool.tile([128, GPT, 128], BF16, tag="px", name="px_t")
            n = cur["n"]
            nc.scalar.activation(
                px_t[:, 0:n, :], ps_t[:, 0:n, :],
                mybir.ActivationFunctionType.Exp, scale=SCALE,
            )
            deferred.append((ps_t, px_t, list(meta)))
            cur["ps"], cur["n"], cur["meta"] = None, 0, []
            if len(deferred) > 3:
                emit_out(deferred.pop(0))

        FILL = 0

        def group(i, r, j):
            """Scores (+mask) for q block i vs k slot (r, j)."""
            if cur["ps"] is None:
                cur["ps"] = ps.tile([128, GPT, 128], F32, tag="ps", name="ps_t")
                # filler matmuls: keep the tensor engine p-state ramped while
                # it waits for exp to free the ring; overwritten by start=True
                for f in range(FILL):
                    nc.tensor.matmul(
                        cur["ps"][:, 2 * f : 2 * f + 2, :],
                        lhsT=ident_sb, rhs=masks_sb[:, :, :],
                        start=True, stop=True,
                    )
            g = cur["n"]
            ps_t = cur["ps"]
            masked = j == i
            nc.tensor.matmul(
                ps_t[:, g, :], lhsT=kT_sb[:, r, j, :], rhs=qT_sb[:, i, :],
                start=True, stop=not masked,
            )
            if masked:
                nc.tensor.matmul(
                    ps_t[:, g, :], lhsT=ident_sb, rhs=masks_sb[:, r, :],
                    start=False, stop=True,
                )
            cur["meta"].append((i, r, j))
            cur["n"] += 1
            if cur["n"] == GPT:
                flush_tile()

        def wave_r(w, r):
            """One rank-half of attention for my q blocks 2w, 2w+1."""
            get_acc(2 * w, r)
            for i in (2 * w, 2 * w + 1):
                for j in range(i + 1):
                    group(i, r, j)

        # ---- pipelined emission ----
        x_dma(0, 256)           # my blocks 0-1
        x_dma(256, 256)         # my blocks 2-3

        # PE warmup: dependency-free matmuls so the tensor engine is at full
        # p-state when proj 0's data lands
        wps = ps.tile([128, GPT, 128], F32, tag="ps", name="wps")
        for wi in range(32):
            nc.tensor.matmul(
                wps[:, wi % GPT, :], lhsT=ident_sb, rhs=ident_sb,
                start=True, stop=True,
            )

        # loop 1: my-half projections + r0 attention phases, fed at one
        # x piece per wave; peer x pieces stream in behind the mine pieces
        # and are consumed by loop 2 while loop 1's compute still runs.
        for p in range(8):
            proj_mine(p)
            wave_r(p, 0)
            if p == 0:
                flush_tile()  # release the first groups early
            if p < 6:
                x_dma(256 * (p + 2), 256)
        for p in range(8):
            x_dma(S // 2 + 256 * p, 256)
        for p in range(8):
            proj_peer(p)
            wave_r(p, 1)
        flush_tile()
        while deferred:
            emit_out(deferred.pop(0))

    nc.finalize()
    return nc


_PROGRAM_CACHE = {}


def _get_program():
    if "prog" not in _PROGRAM_CACHE:
        _PROGRAM_CACHE["prog"] = build_program()
    return _PROGRAM_CACHE["prog"]


def _bf16(a):
    import ml_dtypes
    return np.asarray(a).astype(ml_dtypes.bfloat16)


def _make_masks(parity: int) -> np.ndarray:
    """[2, 128, 128] additive mask tiles for the j == i k slot.

    Slot (0, i) is my own block i = the diagonal (strict lower triangle of
    scores^T masked: k row p > q col j).  Slot (1, i) is the peer's block i:
    natural 2i+1 > 2i for even cores (fully masked), natural 2i < 2i+1 for
    odd cores (fully valid).
    """
    p = np.arange(128)[:, None]
    j = np.arange(128)[None, :]
    tri = np.where(p > j, MASK_VAL, 0.0).astype(np.float32)
    full = np.full((128, 128), MASK_VAL, np.float32)
    zero = np.zeros((128, 128), np.float32)
    m = np.stack([tri, full if parity == 0 else zero])
    return _bf16(m)


def kernel(x, Wq, bq, Wk, bk, Wv, bv):
    x = np.asarray(x, dtype=np.float32)
    wall = np.zeros((D, 256), np.float32)
    wall[:, 0:H] = np.asarray(Wq)
    wall[:, H : 2 * H] = np.asarray(Wk)
    wall[:, 2 * H : 3 * H] = np.asarray(Wv)
    wall = _bf16(wall)
    bqk = np.concatenate(
        [np.asarray(bq), np.asarray(bk)]
    ).astype(np.float32).reshape(2 * H, 1)
    bv_ = np.asarray(bv, dtype=np.float32).reshape(1, H)
    ident = _bf16(np.eye(128, dtype=np.float32))

    nc = _get_program()

    in_maps = []
    for core in range(NCORES):
        b, parity = core // 2, core % 2
        xb = x[b].T.reshape(D, 32, 128)
        mine = xb[:, parity::2, :].reshape(D, S // 2)
        peer = xb[:, 1 - parity :: 2, :].reshape(D, S // 2)
        xTp = np.ascontiguousarray(np.concatenate([mine, peer], axis=1))
        in_maps.append(
            {
                "xT": _bf16(xTp),
                "wall": wall,
                "bqk": bqk,
                "bv": bv_,
                "masks": _make_masks(parity),
                "ident": ident,
            }
        )

    res = run_bass_kernel_spmd(nc, in_maps, list(range(NCORES)))

    out = np.empty((B, S, H), np.float32)
    for core in range(NCORES):
        b, parity = core // 2, core % 2
        o = np.asarray(res.results[core]["out"], np.float32).reshape(NMYB, 128, H)
        out[b].reshape(32, 128, H)[parity::2] = o
    return out
